# revision 26
# baseline (speedup 1.0000x reference)
"""Bass/Trainium2 kernel for nn_BasicBlock_73933567033945 (CDConv / gnn_message_passing).

v2 strategy (graph = fixed +-8 sequence window inside 4 chains, verified at
runtime): shard 8192 nodes across 8 cores (1024 each, half a chain), slot
layout of 128-row halo windows at stride 112.  All matmuls and DVE tensor ops
run in fp16 (fp32 PSUM accumulation); pos is slot-centered on host so fp16
holds precision.  The 17 window shifts are materialized once per core by 17
wide shift-matmuls over all 10 slots (h|pos|ori, 440 cols each).  The
per-edge kernel MLP output is written pair-duplicated (kern2) so the
bilinear kern (x) h product runs in the DVE 2x perf mode.  The (offset,
channel) contraction runs on the PE via PSUM-accumulated transposes followed
by Wk-chunk matmuls, all fp16.  Pure data parallel: no collectives.
"""
import numpy as np

B, L, C = 4, 2048, 128
N = B * L
W = 32
KC = 24
SEQ_L = 11
R = 12.0
WIN = 8
NEG_IN = 0.1
NEG_K = 0.2
NCORES = 8
NPC = N // NCORES          # 1024 nodes per core
TS = 112                   # output nodes per tile
NT = 10                    # tiles per core (9*112 + 16)
HR = 9 * TS + 128          # 1136 halo rows per core
K17 = 2 * WIN + 1          # 17 window offsets
S_HALF = SEQ_L // 2
PH = 44                    # phys cols per slot: h(32) | pos(3) | ori(9)
NBW = NT * PH              # 440: NB cols per k

_PROG = {}


def _sidx(k):
    return int(np.clip(k - WIN, -S_HALF, S_HALF)) + S_HALF


def _build_program():
    import concourse.tile as tile
    from concourse import mybir, bacc
    from concourse.bass_utils import run_bass_kernel_spmd  # noqa: F401 (import check)
    from contextlib import ExitStack

    f32 = mybir.dt.float32
    f16 = mybir.dt.float16
    AF = mybir.ActivationFunctionType
    OP = mybir.AluOpType
    AX = mybir.AxisListType

    nc = bacc.Bacc("TRN2", target_bir_lowering=False, debug=False)

    def din(name, shape, dt=f16):
        return nc.dram_tensor(name, shape, dt, kind="ExternalInput").ap()

    xT_slot = din("xT_slot", [128, NT * 128], f32)   # x transposed per slot
    xc_slot = din("xc_slot", [128, NT * C])          # identity (center rows) f16
    pos_slot = din("pos_slot", [128, NT * 3])        # centered fp16 pos
    ori_slot = din("ori_slot", [128, NT * 9])
    w_in = din("w_in", [C, W])
    ws2a = din("ws2a", [128, 2 * K17 * KC])
    ws2b = din("ws2b", [8, 2 * K17 * KC])
    wk_p = din("wk_p", [128, 6 * W])
    w_out = din("w_out", [W, C])
    ident = din("ident", [128, 128])
    shifts = din("shifts", [128, K17 * TS])
    maskd = din("maskd", [128, NT * K17 * 8])        # expanded to (k, 8)
    kself2 = din("kself2", [128, NT * 2 * KC])
    y = nc.dram_tensor("y", [NPC, C], f32, kind="ExternalOutput").ap()

    P = TS  # 112 active partitions

    with tile.TileContext(nc) as tc, ExitStack() as ctx:
        pers = ctx.enter_context(tc.tile_pool(name="pers", bufs=1))

        def load(ap_in, shape, tag, dt=f16):
            t = pers.tile(shape, dt, tag=tag)
            nc.sync.dma_start(t[:], ap_in)
            return t

        xT_all = load(xT_slot, [128, NT * 128], "xT_all", f32)
        xc_all = load(xc_slot, [128, NT * C], "xc_all")
        w_in_sb = load(w_in, [C, W], "w_in")
        ws2a_sb = load(ws2a, [128, 2 * K17 * KC], "ws2a")
        ws2b_sb = load(ws2b, [8, 2 * K17 * KC], "ws2b")
        wk_sb = load(wk_p, [128, 6 * W], "wk")
        w_out_sb = load(w_out, [W, C], "w_out")
        id_sb = load(ident, [128, 128], "ident")
        sh_sb = load(shifts, [128, K17 * TS], "shifts")
        mask_sb = load(maskd, [128, NT * K17 * 8], "mask")
        ks2_sb = load(kself2, [128, NT * 2 * KC], "kself2")

        # dist = sqrt(d2 + eps): eps = 1e-4 keeps rec = 1/dist <= 100 (fp16
        # safe; self-edges have D = 0 so local = 0 regardless) while real
        # edge distances (>= ~0.5) are perturbed by < 1e-3 relative.
        eps_sb = pers.tile([128, 1], f32, tag="eps")
        nc.vector.memset(eps_sb[:], 1e-4)

        # phys: per slot j, 44 cols [h(32) | pos(3) | ori(9)], all fp16
        phys = pers.tile([128, NBW], f16, tag="phys")
        nc.sync.dma_start(
            phys[:].rearrange("p (j c) -> p j c", c=PH)[:, :, 32:35],
            pos_slot.rearrange("p (j c) -> p j c", c=3))
        nc.sync.dma_start(
            phys[:].rearrange("p (j c) -> p j c", c=PH)[:, :, 35:44],
            ori_slot.rearrange("p (j c) -> p j c", c=9))

        # ---------------- Phase A: h = lrelu(lrelu(x) @ W_in) per slot -----
        with tc.tile_pool(name="pA", bufs=2) as pA, \
             tc.tile_pool(name="pAp", bufs=2, space="PSUM") as pAp:
            for j in range(NT):
                xlT = pA.tile([128, 128], f16, tag="xlT")
                nc.scalar.activation(xlT[:], xT_all[:, 128 * j:128 * (j + 1)],
                                     AF.Prelu, bias=0.0, scale=1.0, alpha=NEG_IN)
                hp = pAp.tile([128, W], f32, tag="hp")
                nc.tensor.matmul(hp[:], xlT[:], w_in_sb[:], start=True, stop=True)
                nc.scalar.activation(phys[:, PH * j:PH * j + W], hp[:],
                                     AF.Prelu, bias=0.0, scale=1.0, alpha=NEG_IN)

        # ---------------- Phase NB: 17 shift matmuls over all slots --------
        NB = pers.tile([P, K17 * NBW], f16, tag="NB")
        with tc.tile_pool(name="pNB", bufs=3, space="PSUM") as pNB:
            for k in range(K17):
                nb_p = pNB.tile([P, NBW], f32, tag="nb_p")
                nc.tensor.matmul(nb_p[:], sh_sb[:, TS * k:TS * (k + 1)],
                                 phys[:], start=True, stop=True)
                dst = NB[:, NBW * k:NBW * (k + 1)]
                if k % 2 == 0:
                    nc.scalar.copy(dst, nb_p[:])
                else:
                    nc.vector.tensor_copy(dst, nb_p[:])

        def nbv(k, t, off, width):
            return NB[:, NBW * k + PH * t + off:NBW * k + PH * t + off + width]

        # ---------------- Phase B: per output tile ------------------------
        wrk = ctx.enter_context(tc.tile_pool(name="wrk", bufs=2))
        tpool = ctx.enter_context(tc.tile_pool(name="tmp", bufs=4))
        psA = ctx.enter_context(tc.tile_pool(name="psA", bufs=2, space="PSUM"))
        psD = ctx.enter_context(tc.tile_pool(name="psD", bufs=1, space="PSUM"))
        psP = ctx.enter_context(tc.tile_pool(name="psP", bufs=1, space="PSUM"))
        psC = ctx.enter_context(tc.tile_pool(name="psC", bufs=1, space="PSUM"))

        for t in range(NT):
            # k-strided views into NB for slot t
            def kview(off, width):
                # [P, K17, width] with k stride NBW
                v = NB[:].rearrange("p (k j) -> p k j", j=NBW)
                return v[:, :, PH * t + off:PH * t + off + width]

            pos_c = nbv(8, t, 32, 3)        # [P, 3] center pos
            ori_c = nbv(8, t, 35, 9)        # [P, 9] center frame

            # ---- geometry -> dav [P, (k,8)] fp16 -------------------------
            D = wrk.tile([P, K17 * 3], f16, tag="D")
            Dv = D[:].rearrange("p (k a) -> p k a", a=3)
            nc.vector.tensor_sub(Dv, kview(32, 3),
                                 pos_c.unsqueeze(1).broadcast_to([P, K17, 3]))
            sq = wrk.tile([P, K17 * 3], f16, tag="sq")
            nc.vector.tensor_mul(sq[:], D[:], D[:])
            d2 = wrk.tile([P, K17], f32, tag="d2")
            nc.vector.tensor_reduce(d2[:], sq[:].rearrange("p (k a) -> p k a", a=3),
                                    axis=AX.X, op=OP.add)
            dav = wrk.tile([P, K17 * 8], f16, tag="dav")
            davv = dav[:].rearrange("p (k d) -> p k d", d=8)
            # dist/R into delta slot 6 (sqrt(d2)/R)
            nc.scalar.activation(davv[:, :, 6], d2[:], AF.Sqrt, bias=0.0,
                                 scale=1.0 / (R * R))
            dist = wrk.tile([P, K17], f32, tag="dist")
            nc.scalar.activation(dist[:], d2[:], AF.Sqrt, bias=eps_sb[0:P, 0:1],
                                 scale=1.0)
            rec = wrk.tile([P, K17], f16, tag="rec")
            with nc.allow_low_precision(reason="fp16 direction scale is ok"):
                nc.vector.reciprocal(rec[:], dist[:])
            # local_a = (sum_b Ri[a,b] * D[k,b]) * rec[k]
            lm = wrk.tile([P, K17 * 9], f16, tag="lm")
            lmv = lm[:].rearrange("p (k a b) -> p k a b", a=3, b=3)
            nc.vector.tensor_mul(
                lmv,
                ori_c.rearrange("p (a b) -> p a b", b=3).unsqueeze(1)
                     .broadcast_to([P, K17, 3, 3]),
                D[:].rearrange("p (k b) -> p k b", b=3).unsqueeze(2)
                    .broadcast_to([P, K17, 3, 3]))
            locr = wrk.tile([P, K17 * 3], f16, tag="locr")
            with nc.allow_low_precision(reason="3-term sums, fp16 ok"):
                nc.vector.tensor_reduce(
                    locr[:].rearrange("p (k a) -> p k a", a=3), lmv,
                    axis=AX.X, op=OP.add)
            nc.vector.tensor_mul(
                davv[:, :, 0:3], locr[:].rearrange("p (k a) -> p k a", a=3),
                rec[:].unsqueeze(-1).broadcast_to([P, K17, 3]))
            # ofeat_a = sum_b Ri[a,b] * Rj[a,b]
            ofm = wrk.tile([P, K17 * 9], f16, tag="ofm")
            nc.vector.tensor_mul(
                ofm[:].rearrange("p (k e) -> p k e", e=9), kview(35, 9),
                ori_c.unsqueeze(1).broadcast_to([P, K17, 9]))
            with nc.allow_low_precision(reason="3-term sums, fp16 ok"):
                nc.vector.tensor_reduce(
                    davv[:, :, 3:6],
                    ofm[:].rearrange("p (k a b) -> p k a b", a=3, b=3),
                    axis=AX.X, op=OP.add)
            nc.vector.memset(davv[:, :, 7], 1.0)
            # chain-boundary mask (zeroes whole delta rows incl. bias);
            # host-expanded to (k, 8) so all operands are packed (2x mode)
            nc.vector.tensor_mul(
                dav[:], dav[:],
                mask_sb[0:P, K17 * 8 * t:K17 * 8 * (t + 1)])

            # ---- kern2 = lrelu(dav @ WS2, 0.2), pair-duplicated ----------
            dT_p = psD.tile([128, 224], f16, tag="dT")
            nc.tensor.matmul(dT_p[:, 0:P], dav[:, 0:128], id_sb[0:P, 0:P],
                             is_transpose=True, start=True, stop=False,
                             skip_group_check=True)
            nc.tensor.matmul(dT_p[0:8, P:P + P], dav[:, 128:136], id_sb[0:P, 0:P],
                             is_transpose=True, start=False, stop=True,
                             skip_group_check=True)
            dT = wrk.tile([128, 224], f16, tag="dT_sb")
            nc.scalar.copy(dT[:], dT_p[:])
            W2 = 2 * K17 * KC  # 816
            # psum banks are 512 f32 cols: put k-blocks 0..9 at 0:480 (bank 0)
            # and k-blocks 10..16 at 512:848 (bank 1) to avoid bank crossing.
            pre_p = psP.tile([P, 848], f32, tag="pre")
            nc.tensor.matmul(pre_p[:, 0:480], dT[:, 0:P], ws2a_sb[:, 0:480],
                             start=True, stop=False, skip_group_check=True)
            nc.tensor.matmul(pre_p[:, 512:848], dT[:, 0:P], ws2a_sb[:, 480:W2],
                             start=True, stop=False, skip_group_check=True)
            nc.tensor.matmul(pre_p[:, 0:480], dT[0:8, P:P + P], ws2b_sb[:, 0:480],
                             start=False, stop=True, skip_group_check=True)
            nc.tensor.matmul(pre_p[:, 512:848], dT[0:8, P:P + P], ws2b_sb[:, 480:W2],
                             start=False, stop=True, skip_group_check=True)
            kern2 = wrk.tile([P, W2], f16, tag="kern2")
            nc.scalar.activation(kern2[:, 0:480], pre_p[:, 0:480], AF.Prelu,
                                 bias=0.0, scale=1.0, alpha=NEG_K)
            nc.scalar.activation(kern2[:, 480:W2], pre_p[:, 512:848], AF.Prelu,
                                 bias=0.0, scale=1.0, alpha=NEG_K)
            # self-edge compensation (host-precomputed, pair-duplicated)
            K8 = 2 * KC * 8
            nc.vector.tensor_add(kern2[:, K8:K8 + 2 * KC],
                                 kern2[:, K8:K8 + 2 * KC],
                                 ks2_sb[0:P, 2 * KC * t:2 * KC * (t + 1)])

            # ---- bilinear + PE transpose-accumulate ----------------------
            # gpsimd takes the last 3 offsets (issued first so they finish
            # by the time the PE transpose chain reaches them); DVE does the
            # rest in the 2x packed mode.
            aggT_p = psA.tile([128, 768], f32, tag="aggT")
            GPK = (14, 15, 16)
            tms = {}

            def bil_mult(k, eng):
                tm = tpool.tile([P, KC * W], f16, tag=f"tm{k % 4}")
                hv = nbv(k, t, 0, 32).rearrange("p (s two) -> p s two", two=2) \
                    .unsqueeze(1).broadcast_to([P, KC, 16, 2])
                kv = kern2[:, 2 * KC * k:2 * KC * (k + 1)] \
                    .rearrange("p (c two) -> p c two", two=2) \
                    .unsqueeze(2).broadcast_to([P, KC, 16, 2])
                eng.tensor_tensor(
                    tm[:].rearrange("p (c s two) -> p c s two", two=2, s=16),
                    hv, kv, op=OP.mult)
                return tm

            for k in GPK:
                tms[k] = bil_mult(k, nc.gpsimd)
            for k in range(K17):
                if k not in GPK:
                    tms[k] = bil_mult(k, nc.vector)
                tm = tms[k]
                for b in range(6):
                    nc.tensor.matmul(
                        aggT_p[:, 128 * b:128 * b + P],
                        tm[:, 128 * b:128 * (b + 1)], id_sb[0:P, 0:P],
                        start=(k == 0 and b in (0, 4)),
                        stop=(k == 16 and b in (3, 5)),
                        skip_group_check=True)
            aggT = wrk.tile([128, 768], f16, tag="aggT_sb")
            nc.scalar.copy(aggT[:], aggT_p[:])

            # ---- conv = lrelu(agg @ Wk, 0.1) ; out = conv @ W_out + x ----
            co_p = psC.tile([P, 240], f32, tag="co")
            for b in range(6):
                nc.tensor.matmul(co_p[0:W, 0:P], wk_sb[:, W * b:W * (b + 1)],
                                 aggT[:, 128 * b:128 * b + P],
                                 start=(b == 0), stop=(b == 5),
                                 skip_group_check=True)
            convL = wrk.tile([W, P], f16, tag="convL")
            nc.scalar.activation(convL[:], co_p[0:W, 0:P], AF.Prelu, bias=0.0,
                                 scale=1.0, alpha=NEG_IN)
            nc.tensor.matmul(co_p[:, P:P + 128], convL[:], w_out_sb[:],
                             start=True, stop=False, skip_group_check=True)
            # identity add on the PE: accumulate xc into the same psum group
            # via an identity-stationary copy-matmul, then DMA from PSUM.
            nc.tensor.matmul(co_p[:, P:P + 128], id_sb[0:P, 0:P],
                             xc_all[0:P, C * t:C * t + C],
                             start=False, stop=True, skip_group_check=True)
            out_sb = wrk.tile([P, C], f32, tag="out_sb")
            nc.scalar.copy(out_sb[:], co_p[:, P:P + 128])
            cnt = min(TS, NPC - TS * t)
            nc.sync.dma_start(y[TS * t:TS * t + cnt, :], out_sb[0:cnt, :])

    nc.compile()
    return nc


def _expected_src_dst():
    i = np.arange(N)
    offs = np.arange(-WIN, WIN + 1)
    j = i[:, None] + offs[None, :]
    valid = ((j // L) == (i[:, None] // L)) & (j >= 0) & (j < N)
    j = np.where(valid, j, i[:, None])
    dst = np.repeat(i, offs.size).astype(np.int32)
    src = j.reshape(-1).astype(np.int32)
    return src, dst


def _host_inputs(x, pos, ori, W_in, Ws0, bs0, Wk, W_out):
    xf = np.ascontiguousarray(x.reshape(N, C), np.float32)
    pos = np.asarray(pos, np.float32)
    ori = np.asarray(ori, np.float32)
    f16 = np.float16

    # shared weights / constants
    WS = np.zeros((136, K17 * KC), np.float32)
    for k in range(K17):
        s = _sidx(k)
        WS[8 * k:8 * k + 7, KC * k:KC * (k + 1)] = Ws0[s]
        WS[8 * k + 7, KC * k:KC * (k + 1)] = bs0[s]
    # pair-duplicate columns: WS2[:, 48k + 2c + j] = WS[:, 24k + c]
    WS2 = np.repeat(WS, 2, axis=1)
    wk_p = np.zeros((128, 6 * W), np.float32)
    for b in range(6):
        wk_p[:, W * b:W * (b + 1)] = Wk[128 * b:128 * (b + 1), :]
    shifts = np.zeros((128, K17 * TS), np.float32)
    for k in range(K17):
        for p in range(TS):
            shifts[p + k, TS * k + p] = 1.0
    common = dict(
        w_in=W_in.astype(f16),
        ws2a=WS2[0:128].astype(f16),
        ws2b=WS2[128:136].astype(f16),
        wk_p=wk_p.astype(f16),
        w_out=W_out.astype(f16),
        ident=np.eye(128, dtype=f16),
        shifts=shifts.astype(f16),
    )

    # self-edge compensation: kself[n] = lrelu(rn @ W5[3:6] + b5, 0.2) * ncl
    rn = (ori.reshape(N, 3, 3) ** 2).sum(axis=2)          # [N, 3]
    pself = rn @ np.asarray(Ws0[S_HALF][3:6], np.float32) \
        + np.asarray(bs0[S_HALF], np.float32)             # [N, KC]
    kself_full = np.where(pself >= 0, pself, NEG_K * pself)

    in_maps = []
    for ci in range(NCORES):
        s0 = ci * NPC
        g = s0 - WIN + np.arange(HR)
        ok = (g >= 0) & (g < N)
        gi = np.clip(g, 0, N - 1)
        x_pad = np.where(ok[:, None], xf[gi], 0.0).astype(np.float32)
        p_pad = np.where(ok[:, None], pos[gi], 0.0).astype(np.float32)
        o_pad = np.where(ok[:, None], ori[gi], 0.0).astype(np.float32)

        jj, pp = np.meshgrid(np.arange(NT), np.arange(128), indexing="ij")
        rows = (TS * jj + pp)            # [NT, 128] all < HR
        # xT_slot: [128(c), (t, p)] transposed slots
        x_sl = x_pad[rows]               # [NT, 128, C]
        xT_slot = np.ascontiguousarray(
            x_sl.transpose(2, 0, 1).reshape(C, NT * 128), np.float32)
        # pos: center per slot for fp16 precision
        p_sl = p_pad[rows]               # [NT, 128, 3]
        ctr = p_sl.mean(axis=1, keepdims=True)
        p_sl = (p_sl - ctr).astype(f16)
        pos_slot = np.ascontiguousarray(
            p_sl.transpose(1, 0, 2).reshape(128, NT * 3))
        o_sl = o_pad[rows].astype(f16)
        ori_slot = np.ascontiguousarray(
            o_sl.transpose(1, 0, 2).reshape(128, NT * 9))
        # identity (center rows)
        rc = WIN + TS * jj + pp
        okc = rc < HR
        xc_slot = np.where(okc[:, :, None], x_pad[np.minimum(rc, HR - 1)], 0.0)
        xc_slot = xc_slot.transpose(1, 0, 2).reshape(128, NT * C).astype(f16)

        # mask + boundary-count + kself2 (output-node indexed)
        mask = np.zeros((128, NT, K17), np.float32)
        ncl = np.zeros((128, NT), np.float32)
        for t in range(NT):
            nvalid = min(TS, NPC - TS * t)
            for p in range(nvalid):
                n = s0 + TS * t + p
                off = n % L
                v = ((off + np.arange(-WIN, WIN + 1)) >= 0) & \
                    ((off + np.arange(-WIN, WIN + 1)) < L)
                mask[p, t, :] = v.astype(np.float32)
                ncl[p, t] = K17 - v.sum()
        ks = np.zeros((128, NT, KC), np.float32)
        for t in range(NT):
            nvalid = min(TS, NPC - TS * t)
            rowsn = s0 + TS * t + np.arange(nvalid)
            ks[:nvalid, t, :] = kself_full[rowsn] * ncl[:nvalid, t][:, None]
        ks2 = np.repeat(ks, 2, axis=2)  # duplicate pairs within each KC block
        in_maps.append(dict(
            xT_slot=xT_slot, xc_slot=xc_slot,
            pos_slot=pos_slot, ori_slot=ori_slot,
            maskd=np.repeat(mask.reshape(128, NT, K17, 1), 8, axis=3)
            .reshape(128, NT * K17 * 8).astype(f16),
            kself2=ks2.reshape(128, NT * 2 * KC).astype(f16),
            **common))
    return in_maps


def kernel(x, pos, seq, ori, W_in, Ws0, bs0, Wk, W_out, src, dst):
    exp_src, exp_dst = _expected_src_dst()
    assert np.array_equal(np.asarray(src), exp_src), "unexpected src graph"
    assert np.array_equal(np.asarray(dst), exp_dst), "unexpected dst graph"

    from concourse.bass_utils import run_bass_kernel_spmd

    if "nc" not in _PROG:
        _PROG["nc"] = _build_program()
    nc = _PROG["nc"]

    in_maps = _host_inputs(np.asarray(x), np.asarray(pos), np.asarray(ori),
                           np.asarray(W_in), np.asarray(Ws0), np.asarray(bs0),
                           np.asarray(Wk), np.asarray(W_out))
    res = run_bass_kernel_spmd(nc, in_maps, list(range(NCORES)))
    out = np.concatenate([res.results[i]["y"] for i in range(NCORES)], axis=0)
    return out.reshape(B, L, C).astype(np.float32)


# revision 28
# speedup vs baseline: 1.0820x; 1.0820x over previous
"""Bass/Trainium2 kernel for nn_BasicBlock_73933567033945 (CDConv / gnn_message_passing).

v2 strategy (graph = fixed +-8 sequence window inside 4 chains, verified at
runtime): shard 8192 nodes across 8 cores (1024 each, half a chain), slot
layout of 128-row halo windows at stride 112.  All matmuls and DVE tensor ops
run in fp16 (fp32 PSUM accumulation); pos is slot-centered on host so fp16
holds precision.  The 17 window shifts are materialized once per core by 17
wide shift-matmuls over all 10 slots (h|pos|ori, 440 cols each).  The
per-edge kernel MLP output is written pair-duplicated (kern2) so the
bilinear kern (x) h product runs in the DVE 2x perf mode.  The (offset,
channel) contraction runs on the PE via PSUM-accumulated transposes followed
by Wk-chunk matmuls, all fp16.  Pure data parallel: no collectives.
"""
import numpy as np

B, L, C = 4, 2048, 128
N = B * L
W = 32
KC = 24
SEQ_L = 11
R = 12.0
WIN = 8
NEG_IN = 0.1
NEG_K = 0.2
NCORES = 8
NPC = N // NCORES          # 1024 nodes per core
TS = 112                   # output nodes per tile
NT = 10                    # tiles per core (9*112 + 16)
HR = 9 * TS + 128          # 1136 halo rows per core
K17 = 2 * WIN + 1          # 17 window offsets
S_HALF = SEQ_L // 2
PH = 44                    # phys cols per slot: h(32) | pos(3) | ori(9)
NBW = NT * PH              # 440: NB cols per k

_PROG = {}


def _sidx(k):
    return int(np.clip(k - WIN, -S_HALF, S_HALF)) + S_HALF


def _build_program():
    import concourse.tile as tile
    from concourse import mybir, bacc
    from concourse.bass_utils import run_bass_kernel_spmd  # noqa: F401 (import check)
    from contextlib import ExitStack

    f32 = mybir.dt.float32
    f16 = mybir.dt.float16
    AF = mybir.ActivationFunctionType
    OP = mybir.AluOpType
    AX = mybir.AxisListType

    nc = bacc.Bacc("TRN2", target_bir_lowering=False, debug=False)

    def din(name, shape, dt=f16):
        return nc.dram_tensor(name, shape, dt, kind="ExternalInput").ap()

    xT_slot = din("xT_slot", [128, NT * 128], f32)   # x transposed per slot
    xc_slot = din("xc_slot", [128, NT * C])          # identity (center rows) f16
    pos_slot = din("pos_slot", [128, NT * 3])        # centered fp16 pos
    ori_slot = din("ori_slot", [128, NT * 9])
    w_in = din("w_in", [C, W])
    ws2a = din("ws2a", [128, 2 * K17 * KC])
    ws2b = din("ws2b", [8, 2 * K17 * KC])
    wk_p = din("wk_p", [128, 6 * W])
    w_out = din("w_out", [W, C])
    ident = din("ident", [128, 128])
    shifts = din("shifts", [128, K17 * TS])
    maskd = din("maskd", [128, NT * K17 * 8])        # expanded to (k, 8)
    kself2 = din("kself2", [128, NT * 2 * KC])
    y = nc.dram_tensor("y", [NPC, C], f32, kind="ExternalOutput").ap()

    P = TS  # 112 active partitions

    with tile.TileContext(nc) as tc, ExitStack() as ctx:
        pers = ctx.enter_context(tc.tile_pool(name="pers", bufs=1))

        def load(ap_in, shape, tag, dt=f16):
            t = pers.tile(shape, dt, tag=tag)
            nc.sync.dma_start(t[:], ap_in)
            return t

        xT_all = load(xT_slot, [128, NT * 128], "xT_all", f32)
        xc_all = load(xc_slot, [128, NT * C], "xc_all")
        w_in_sb = load(w_in, [C, W], "w_in")
        ws2a_sb = load(ws2a, [128, 2 * K17 * KC], "ws2a")
        ws2b_sb = load(ws2b, [8, 2 * K17 * KC], "ws2b")
        wk_sb = load(wk_p, [128, 6 * W], "wk")
        w_out_sb = load(w_out, [W, C], "w_out")
        id_sb = load(ident, [128, 128], "ident")
        sh_sb = load(shifts, [128, K17 * TS], "shifts")
        mask_sb = load(maskd, [128, NT * K17 * 8], "mask")
        ks2_sb = load(kself2, [128, NT * 2 * KC], "kself2")

        # dist = sqrt(d2 + eps): eps = 1e-4 keeps rec = 1/dist <= 100 (fp16
        # safe; self-edges have D = 0 so local = 0 regardless) while real
        # edge distances (>= ~0.5) are perturbed by < 1e-3 relative.
        eps_sb = pers.tile([128, 1], f32, tag="eps")
        nc.vector.memset(eps_sb[:], 1e-4)

        # phys: per slot j, 44 cols [h(32) | pos(3) | ori(9)], all fp16
        phys = pers.tile([128, NBW], f16, tag="phys")
        nc.sync.dma_start(
            phys[:].rearrange("p (j c) -> p j c", c=PH)[:, :, 32:35],
            pos_slot.rearrange("p (j c) -> p j c", c=3))
        nc.sync.dma_start(
            phys[:].rearrange("p (j c) -> p j c", c=PH)[:, :, 35:44],
            ori_slot.rearrange("p (j c) -> p j c", c=9))

        # ---------------- Phase A: h = lrelu(lrelu(x) @ W_in) per slot -----
        with tc.tile_pool(name="pA", bufs=2) as pA, \
             tc.tile_pool(name="pAp", bufs=2, space="PSUM") as pAp:
            for j in range(NT):
                xlT = pA.tile([128, 128], f16, tag="xlT")
                nc.scalar.activation(xlT[:], xT_all[:, 128 * j:128 * (j + 1)],
                                     AF.Prelu, bias=0.0, scale=1.0, alpha=NEG_IN)
                hp = pAp.tile([128, W], f32, tag="hp")
                nc.tensor.matmul(hp[:], xlT[:], w_in_sb[:], start=True, stop=True)
                nc.scalar.activation(phys[:, PH * j:PH * j + W], hp[:],
                                     AF.Prelu, bias=0.0, scale=1.0, alpha=NEG_IN)

        # ---------------- Phase NB: 17 shift matmuls over all slots --------
        NB = pers.tile([P, K17 * NBW], f16, tag="NB")
        with tc.tile_pool(name="pNB", bufs=3, space="PSUM") as pNB:
            for k in range(K17):
                nb_p = pNB.tile([P, NBW], f32, tag="nb_p")
                nc.tensor.matmul(nb_p[:], sh_sb[:, TS * k:TS * (k + 1)],
                                 phys[:], start=True, stop=True)
                nc.scalar.copy(NB[:, NBW * k:NBW * (k + 1)], nb_p[:])

        def nbv(k, t, off, width):
            return NB[:, NBW * k + PH * t + off:NBW * k + PH * t + off + width]

        # ---------------- Phase B: per output tile ------------------------
        wrk = ctx.enter_context(tc.tile_pool(name="wrk", bufs=2))
        tpool = ctx.enter_context(tc.tile_pool(name="tmp", bufs=4))
        psA = ctx.enter_context(tc.tile_pool(name="psA", bufs=2, space="PSUM"))
        psD = ctx.enter_context(tc.tile_pool(name="psD", bufs=1, space="PSUM"))
        psP = ctx.enter_context(tc.tile_pool(name="psP", bufs=1, space="PSUM"))
        psC = ctx.enter_context(tc.tile_pool(name="psC", bufs=1, space="PSUM"))

        for t in range(NT):
            # k-strided views into NB for slot t
            def kview(off, width):
                # [P, K17, width] with k stride NBW
                v = NB[:].rearrange("p (k j) -> p k j", j=NBW)
                return v[:, :, PH * t + off:PH * t + off + width]

            pos_c = nbv(8, t, 32, 3)        # [P, 3] center pos
            ori_c = nbv(8, t, 35, 9)        # [P, 9] center frame

            # ---- geometry -> dav [P, (k,8)] fp16 -------------------------
            D = wrk.tile([P, K17 * 3], f16, tag="D")
            Dv = D[:].rearrange("p (k a) -> p k a", a=3)
            nc.vector.tensor_sub(Dv, kview(32, 3),
                                 pos_c.unsqueeze(1).broadcast_to([P, K17, 3]))
            sq = wrk.tile([P, K17 * 3], f16, tag="sq")
            nc.vector.tensor_mul(sq[:], D[:], D[:])
            d2 = wrk.tile([P, K17], f32, tag="d2")
            nc.vector.tensor_reduce(d2[:], sq[:].rearrange("p (k a) -> p k a", a=3),
                                    axis=AX.X, op=OP.add)
            dav = wrk.tile([P, K17 * 8], f16, tag="dav")
            davv = dav[:].rearrange("p (k d) -> p k d", d=8)
            # dist/R into delta slot 6 (sqrt(d2)/R)
            nc.scalar.activation(davv[:, :, 6], d2[:], AF.Sqrt, bias=0.0,
                                 scale=1.0 / (R * R))
            dist = wrk.tile([P, K17], f32, tag="dist")
            nc.scalar.activation(dist[:], d2[:], AF.Sqrt, bias=eps_sb[0:P, 0:1],
                                 scale=1.0)
            rec = wrk.tile([P, K17], f16, tag="rec")
            with nc.allow_low_precision(reason="fp16 direction scale is ok"):
                nc.vector.reciprocal(rec[:], dist[:])
            # local_a = (sum_b Ri[a,b] * D[k,b]) * rec[k]
            lm = wrk.tile([P, K17 * 9], f16, tag="lm")
            lmv = lm[:].rearrange("p (k a b) -> p k a b", a=3, b=3)
            nc.vector.tensor_mul(
                lmv,
                ori_c.rearrange("p (a b) -> p a b", b=3).unsqueeze(1)
                     .broadcast_to([P, K17, 3, 3]),
                D[:].rearrange("p (k b) -> p k b", b=3).unsqueeze(2)
                    .broadcast_to([P, K17, 3, 3]))
            locr = wrk.tile([P, K17 * 3], f16, tag="locr")
            with nc.allow_low_precision(reason="3-term sums, fp16 ok"):
                nc.vector.tensor_reduce(
                    locr[:].rearrange("p (k a) -> p k a", a=3), lmv,
                    axis=AX.X, op=OP.add)
            nc.vector.tensor_mul(
                davv[:, :, 0:3], locr[:].rearrange("p (k a) -> p k a", a=3),
                rec[:].unsqueeze(-1).broadcast_to([P, K17, 3]))
            # ofeat_a = sum_b Ri[a,b] * Rj[a,b]
            ofm = wrk.tile([P, K17 * 9], f16, tag="ofm")
            nc.vector.tensor_mul(
                ofm[:].rearrange("p (k e) -> p k e", e=9), kview(35, 9),
                ori_c.unsqueeze(1).broadcast_to([P, K17, 9]))
            with nc.allow_low_precision(reason="3-term sums, fp16 ok"):
                nc.vector.tensor_reduce(
                    davv[:, :, 3:6],
                    ofm[:].rearrange("p (k a b) -> p k a b", a=3, b=3),
                    axis=AX.X, op=OP.add)
            nc.vector.memset(davv[:, :, 7], 1.0)
            # chain-boundary mask (zeroes whole delta rows incl. bias);
            # host-expanded to (k, 8) so all operands are packed (2x mode)
            nc.vector.tensor_mul(
                dav[:], dav[:],
                mask_sb[0:P, K17 * 8 * t:K17 * 8 * (t + 1)])

            # ---- kern2 = lrelu(dav @ WS2, 0.2), pair-duplicated ----------
            dT_p = psD.tile([128, 224], f16, tag="dT")
            nc.tensor.matmul(dT_p[:, 0:P], dav[:, 0:128], id_sb[0:P, 0:P],
                             is_transpose=True, start=True, stop=False,
                             skip_group_check=True)
            nc.tensor.matmul(dT_p[0:8, P:P + P], dav[:, 128:136], id_sb[0:P, 0:P],
                             is_transpose=True, start=False, stop=True,
                             skip_group_check=True)
            dT = wrk.tile([128, 224], f16, tag="dT_sb")
            nc.scalar.copy(dT[:], dT_p[:])
            W2 = 2 * K17 * KC  # 816
            # psum banks are 512 f32 cols: put k-blocks 0..9 at 0:480 (bank 0)
            # and k-blocks 10..16 at 512:848 (bank 1) to avoid bank crossing.
            pre_p = psP.tile([P, 848], f32, tag="pre")
            nc.tensor.matmul(pre_p[:, 0:480], dT[:, 0:P], ws2a_sb[:, 0:480],
                             start=True, stop=False, skip_group_check=True)
            nc.tensor.matmul(pre_p[:, 512:848], dT[:, 0:P], ws2a_sb[:, 480:W2],
                             start=True, stop=False, skip_group_check=True)
            nc.tensor.matmul(pre_p[:, 0:480], dT[0:8, P:P + P], ws2b_sb[:, 0:480],
                             start=False, stop=True, skip_group_check=True)
            nc.tensor.matmul(pre_p[:, 512:848], dT[0:8, P:P + P], ws2b_sb[:, 480:W2],
                             start=False, stop=True, skip_group_check=True)
            kern2 = wrk.tile([P, W2], f16, tag="kern2")
            nc.scalar.activation(kern2[:, 0:480], pre_p[:, 0:480], AF.Prelu,
                                 bias=0.0, scale=1.0, alpha=NEG_K)
            nc.scalar.activation(kern2[:, 480:W2], pre_p[:, 512:848], AF.Prelu,
                                 bias=0.0, scale=1.0, alpha=NEG_K)
            # self-edge compensation (host-precomputed, pair-duplicated)
            K8 = 2 * KC * 8
            nc.vector.tensor_add(kern2[:, K8:K8 + 2 * KC],
                                 kern2[:, K8:K8 + 2 * KC],
                                 ks2_sb[0:P, 2 * KC * t:2 * KC * (t + 1)])

            # ---- bilinear + PE transpose-accumulate ----------------------
            # gpsimd takes the last 3 offsets (issued first so they finish
            # by the time the PE transpose chain reaches them); DVE does the
            # rest in the 2x packed mode.
            aggT_p = psA.tile([128, 768], f32, tag="aggT")
            for k in range(K17):
                tm = tpool.tile([P, KC * W], f16, tag="tm")
                hv = nbv(k, t, 0, 32).rearrange("p (s two) -> p s two", two=2) \
                    .unsqueeze(1).broadcast_to([P, KC, 16, 2])
                kv = kern2[:, 2 * KC * k:2 * KC * (k + 1)] \
                    .rearrange("p (c two) -> p c two", two=2) \
                    .unsqueeze(2).broadcast_to([P, KC, 16, 2])
                nc.vector.tensor_tensor(
                    tm[:].rearrange("p (c s two) -> p c s two", two=2, s=16),
                    hv, kv, op=OP.mult)
                for b in range(6):
                    nc.tensor.matmul(
                        aggT_p[:, 128 * b:128 * b + P],
                        tm[:, 128 * b:128 * (b + 1)], id_sb[0:P, 0:P],
                        start=(k == 0 and b in (0, 4)),
                        stop=(k == 16 and b in (3, 5)),
                        skip_group_check=True)
            aggT = wrk.tile([128, 768], f16, tag="aggT_sb")
            nc.scalar.copy(aggT[:], aggT_p[:])

            # ---- conv = lrelu(agg @ Wk, 0.1) ; out = conv @ W_out + x ----
            co_p = psC.tile([P, 240], f32, tag="co")
            for b in range(6):
                nc.tensor.matmul(co_p[0:W, 0:P], wk_sb[:, W * b:W * (b + 1)],
                                 aggT[:, 128 * b:128 * b + P],
                                 start=(b == 0), stop=(b == 5),
                                 skip_group_check=True)
            convL = wrk.tile([W, P], f16, tag="convL")
            nc.scalar.activation(convL[:], co_p[0:W, 0:P], AF.Prelu, bias=0.0,
                                 scale=1.0, alpha=NEG_IN)
            nc.tensor.matmul(co_p[:, P:P + 128], convL[:], w_out_sb[:],
                             start=True, stop=False, skip_group_check=True)
            # identity add on the PE: accumulate xc into the same psum group
            # via an identity-stationary copy-matmul, then DMA from PSUM.
            nc.tensor.matmul(co_p[:, P:P + 128], id_sb[0:P, 0:P],
                             xc_all[0:P, C * t:C * t + C],
                             start=False, stop=True, skip_group_check=True)
            out_sb = wrk.tile([P, C], f32, tag="out_sb")
            nc.scalar.copy(out_sb[:], co_p[:, P:P + 128])
            cnt = min(TS, NPC - TS * t)
            nc.sync.dma_start(y[TS * t:TS * t + cnt, :], out_sb[0:cnt, :])

    nc.compile()
    return nc


def _expected_src_dst():
    i = np.arange(N)
    offs = np.arange(-WIN, WIN + 1)
    j = i[:, None] + offs[None, :]
    valid = ((j // L) == (i[:, None] // L)) & (j >= 0) & (j < N)
    j = np.where(valid, j, i[:, None])
    dst = np.repeat(i, offs.size).astype(np.int32)
    src = j.reshape(-1).astype(np.int32)
    return src, dst


def _host_inputs(x, pos, ori, W_in, Ws0, bs0, Wk, W_out):
    xf = np.ascontiguousarray(x.reshape(N, C), np.float32)
    pos = np.asarray(pos, np.float32)
    ori = np.asarray(ori, np.float32)
    f16 = np.float16

    # shared weights / constants
    WS = np.zeros((136, K17 * KC), np.float32)
    for k in range(K17):
        s = _sidx(k)
        WS[8 * k:8 * k + 7, KC * k:KC * (k + 1)] = Ws0[s]
        WS[8 * k + 7, KC * k:KC * (k + 1)] = bs0[s]
    # pair-duplicate columns: WS2[:, 48k + 2c + j] = WS[:, 24k + c]
    WS2 = np.repeat(WS, 2, axis=1)
    wk_p = np.zeros((128, 6 * W), np.float32)
    for b in range(6):
        wk_p[:, W * b:W * (b + 1)] = Wk[128 * b:128 * (b + 1), :]
    shifts = np.zeros((128, K17 * TS), np.float32)
    for k in range(K17):
        for p in range(TS):
            shifts[p + k, TS * k + p] = 1.0
    common = dict(
        w_in=W_in.astype(f16),
        ws2a=WS2[0:128].astype(f16),
        ws2b=WS2[128:136].astype(f16),
        wk_p=wk_p.astype(f16),
        w_out=W_out.astype(f16),
        ident=np.eye(128, dtype=f16),
        shifts=shifts.astype(f16),
    )

    # self-edge compensation: kself[n] = lrelu(rn @ W5[3:6] + b5, 0.2) * ncl
    rn = (ori.reshape(N, 3, 3) ** 2).sum(axis=2)          # [N, 3]
    pself = rn @ np.asarray(Ws0[S_HALF][3:6], np.float32) \
        + np.asarray(bs0[S_HALF], np.float32)             # [N, KC]
    kself_full = np.where(pself >= 0, pself, NEG_K * pself)

    in_maps = []
    for ci in range(NCORES):
        s0 = ci * NPC
        g = s0 - WIN + np.arange(HR)
        ok = (g >= 0) & (g < N)
        gi = np.clip(g, 0, N - 1)
        x_pad = np.where(ok[:, None], xf[gi], 0.0).astype(np.float32)
        p_pad = np.where(ok[:, None], pos[gi], 0.0).astype(np.float32)
        o_pad = np.where(ok[:, None], ori[gi], 0.0).astype(np.float32)

        jj, pp = np.meshgrid(np.arange(NT), np.arange(128), indexing="ij")
        rows = (TS * jj + pp)            # [NT, 128] all < HR
        # xT_slot: [128(c), (t, p)] transposed slots
        x_sl = x_pad[rows]               # [NT, 128, C]
        xT_slot = np.ascontiguousarray(
            x_sl.transpose(2, 0, 1).reshape(C, NT * 128), np.float32)
        # pos: center per slot for fp16 precision
        p_sl = p_pad[rows]               # [NT, 128, 3]
        ctr = p_sl.mean(axis=1, keepdims=True)
        p_sl = (p_sl - ctr).astype(f16)
        pos_slot = np.ascontiguousarray(
            p_sl.transpose(1, 0, 2).reshape(128, NT * 3))
        o_sl = o_pad[rows].astype(f16)
        ori_slot = np.ascontiguousarray(
            o_sl.transpose(1, 0, 2).reshape(128, NT * 9))
        # identity (center rows)
        rc = WIN + TS * jj + pp
        okc = rc < HR
        xc_slot = np.where(okc[:, :, None], x_pad[np.minimum(rc, HR - 1)], 0.0)
        xc_slot = xc_slot.transpose(1, 0, 2).reshape(128, NT * C).astype(f16)

        # mask + boundary-count + kself2 (output-node indexed)
        mask = np.zeros((128, NT, K17), np.float32)
        ncl = np.zeros((128, NT), np.float32)
        for t in range(NT):
            nvalid = min(TS, NPC - TS * t)
            for p in range(nvalid):
                n = s0 + TS * t + p
                off = n % L
                v = ((off + np.arange(-WIN, WIN + 1)) >= 0) & \
                    ((off + np.arange(-WIN, WIN + 1)) < L)
                mask[p, t, :] = v.astype(np.float32)
                ncl[p, t] = K17 - v.sum()
        ks = np.zeros((128, NT, KC), np.float32)
        for t in range(NT):
            nvalid = min(TS, NPC - TS * t)
            rowsn = s0 + TS * t + np.arange(nvalid)
            ks[:nvalid, t, :] = kself_full[rowsn] * ncl[:nvalid, t][:, None]
        ks2 = np.repeat(ks, 2, axis=2)  # duplicate pairs within each KC block
        in_maps.append(dict(
            xT_slot=xT_slot, xc_slot=xc_slot,
            pos_slot=pos_slot, ori_slot=ori_slot,
            maskd=np.repeat(mask.reshape(128, NT, K17, 1), 8, axis=3)
            .reshape(128, NT * K17 * 8).astype(f16),
            kself2=ks2.reshape(128, NT * 2 * KC).astype(f16),
            **common))
    return in_maps


def kernel(x, pos, seq, ori, W_in, Ws0, bs0, Wk, W_out, src, dst):
    exp_src, exp_dst = _expected_src_dst()
    assert np.array_equal(np.asarray(src), exp_src), "unexpected src graph"
    assert np.array_equal(np.asarray(dst), exp_dst), "unexpected dst graph"

    from concourse.bass_utils import run_bass_kernel_spmd

    if "nc" not in _PROG:
        _PROG["nc"] = _build_program()
    nc = _PROG["nc"]

    in_maps = _host_inputs(np.asarray(x), np.asarray(pos), np.asarray(ori),
                           np.asarray(W_in), np.asarray(Ws0), np.asarray(bs0),
                           np.asarray(Wk), np.asarray(W_out))
    res = run_bass_kernel_spmd(nc, in_maps, list(range(NCORES)))
    out = np.concatenate([res.results[i]["y"] for i in range(NCORES)], axis=0)
    return out.reshape(B, L, C).astype(np.float32)


# revision 33
# speedup vs baseline: 1.1148x; 1.0303x over previous
"""Bass/Trainium2 kernel for nn_BasicBlock_73933567033945 (CDConv / gnn_message_passing).

v2 strategy (graph = fixed +-8 sequence window inside 4 chains, verified at
runtime): shard 8192 nodes across 8 cores (1024 each, half a chain), slot
layout of 128-row halo windows at stride 112.  All matmuls and DVE tensor ops
run in fp16 (fp32 PSUM accumulation); pos is slot-centered on host so fp16
holds precision.  The 17 window shifts are materialized once per core by 17
wide shift-matmuls over all 10 slots (h|pos|ori, 440 cols each).  The
per-edge kernel MLP output is written pair-duplicated (kern2) so the
bilinear kern (x) h product runs in the DVE 2x perf mode.  The (offset,
channel) contraction runs on the PE via PSUM-accumulated transposes followed
by Wk-chunk matmuls, all fp16.  Pure data parallel: no collectives.
"""
import numpy as np

B, L, C = 4, 2048, 128
N = B * L
W = 32
KC = 24
SEQ_L = 11
R = 12.0
WIN = 8
NEG_IN = 0.1
NEG_K = 0.2
NCORES = 8
NPC = N // NCORES          # 1024 nodes per core
TS = 112                   # output nodes per tile
NT = 10                    # tiles per core (9*112 + 16)
HR = 9 * TS + 128          # 1136 halo rows per core
K17 = 2 * WIN + 1          # 17 window offsets
S_HALF = SEQ_L // 2
PH = 44                    # phys cols per slot: h(32) | pos(3) | ori(9)
NBW = NT * PH              # 440: NB cols per k

_PROG = {}


def _sidx(k):
    return int(np.clip(k - WIN, -S_HALF, S_HALF)) + S_HALF


def _build_program():
    import concourse.tile as tile
    from concourse import mybir, bacc
    from concourse.bass_utils import run_bass_kernel_spmd  # noqa: F401 (import check)
    from contextlib import ExitStack

    f32 = mybir.dt.float32
    f16 = mybir.dt.float16
    AF = mybir.ActivationFunctionType
    OP = mybir.AluOpType
    AX = mybir.AxisListType

    nc = bacc.Bacc("TRN2", target_bir_lowering=False, debug=False)

    def din(name, shape, dt=f16):
        return nc.dram_tensor(name, shape, dt, kind="ExternalInput").ap()

    xT_slot = din("xT_slot", [128, NT * 128])        # x transposed per slot, f16
    xc_slot = din("xc_slot", [128, NT * C])          # identity (center rows) f16
    pos_slot = din("pos_slot", [128, NT * 3])        # centered fp16 pos
    ori_slot = din("ori_slot", [128, NT * 9])
    w_in = din("w_in", [C, W])
    ws2a = din("ws2a", [128, 2 * K17 * KC])
    ws2b = din("ws2b", [8, 2 * K17 * KC])
    wk_p = din("wk_p", [128, 6 * W])
    w_out = din("w_out", [W, C])
    ident = din("ident", [128, 128])
    shifts = din("shifts", [128, K17 * TS])
    maskd = din("maskd", [128, NT * K17 * 8])        # expanded to (k, 8)
    kself2 = din("kself2", [128, NT * 2 * KC])
    y = nc.dram_tensor("y", [NPC, C], f32, kind="ExternalOutput").ap()

    P = TS  # 112 active partitions

    with tile.TileContext(nc) as tc, ExitStack() as ctx:
        pers = ctx.enter_context(tc.tile_pool(name="pers", bufs=1))

        def load(ap_in, shape, tag, dt=f16):
            t = pers.tile(shape, dt, tag=tag)
            nc.sync.dma_start(t[:], ap_in)
            return t

        xT_all = load(xT_slot, [128, NT * 128], "xT_all")
        xc_all = load(xc_slot, [128, NT * C], "xc_all")
        w_in_sb = load(w_in, [C, W], "w_in")
        ws2a_sb = load(ws2a, [128, 2 * K17 * KC], "ws2a")
        ws2b_sb = load(ws2b, [8, 2 * K17 * KC], "ws2b")
        wk_sb = load(wk_p, [128, 6 * W], "wk")
        w_out_sb = load(w_out, [W, C], "w_out")
        id_sb = load(ident, [128, 128], "ident")
        sh_sb = load(shifts, [128, K17 * TS], "shifts")
        mask_sb = load(maskd, [128, NT * K17 * 8], "mask")
        ks2_sb = load(kself2, [128, NT * 2 * KC], "kself2")

        # dist = sqrt(d2 + eps): eps = 1e-4 keeps rec = 1/dist <= 100 (fp16
        # safe; self-edges have D = 0 so local = 0 regardless) while real
        # edge distances (>= ~0.5) are perturbed by < 1e-3 relative.
        eps_sb = pers.tile([128, 1], f32, tag="eps")
        nc.vector.memset(eps_sb[:], 1e-4)

        # phys: per slot j, 44 cols [h(32) | pos(3) | ori(9)], all fp16
        phys = pers.tile([128, NBW], f16, tag="phys")
        nc.sync.dma_start(
            phys[:].rearrange("p (j c) -> p j c", c=PH)[:, :, 32:35],
            pos_slot.rearrange("p (j c) -> p j c", c=3))
        nc.sync.dma_start(
            phys[:].rearrange("p (j c) -> p j c", c=PH)[:, :, 35:44],
            ori_slot.rearrange("p (j c) -> p j c", c=9))

        # ---------------- Phase A: h = lrelu(lrelu(x) @ W_in) per slot -----
        with tc.tile_pool(name="pA", bufs=2) as pA, \
             tc.tile_pool(name="pAp", bufs=2, space="PSUM") as pAp:
            for j in range(NT):
                xlT = pA.tile([128, 128], f16, tag="xlT")
                nc.scalar.activation(xlT[:], xT_all[:, 128 * j:128 * (j + 1)],
                                     AF.Prelu, bias=0.0, scale=1.0, alpha=NEG_IN)
                hp = pAp.tile([128, W], f32, tag="hp")
                nc.tensor.matmul(hp[:], xlT[:], w_in_sb[:], start=True, stop=True)
                nc.scalar.activation(phys[:, PH * j:PH * j + W], hp[:],
                                     AF.Prelu, bias=0.0, scale=1.0, alpha=NEG_IN)

        # ---------------- Phase NB: 17 shift matmuls over all slots --------
        NB = pers.tile([P, K17 * NBW], f16, tag="NB")
        with tc.tile_pool(name="pNB", bufs=3, space="PSUM") as pNB:
            for k in range(K17):
                nb_p = pNB.tile([P, NBW], f32, tag="nb_p")
                nc.tensor.matmul(nb_p[:], sh_sb[:, TS * k:TS * (k + 1)],
                                 phys[:], start=True, stop=True)
                nc.scalar.copy(NB[:, NBW * k:NBW * (k + 1)], nb_p[:])

        def nbv(k, t, off, width):
            return NB[:, NBW * k + PH * t + off:NBW * k + PH * t + off + width]

        # ---------------- Phase B: per output tile ------------------------
        wrk = ctx.enter_context(tc.tile_pool(name="wrk", bufs=2))
        tpool = ctx.enter_context(tc.tile_pool(name="tmp", bufs=4))
        psG = ctx.enter_context(tc.tile_pool(name="psG", bufs=1, space="PSUM"))
        psA = ctx.enter_context(tc.tile_pool(name="psA", bufs=1, space="PSUM"))
        psD = ctx.enter_context(tc.tile_pool(name="psD", bufs=1, space="PSUM"))
        psP = ctx.enter_context(tc.tile_pool(name="psP", bufs=1, space="PSUM"))
        psC = ctx.enter_context(tc.tile_pool(name="psC", bufs=1, space="PSUM"))

        for t in range(NT):
            # k-strided views into NB for slot t
            def kview(off, width):
                # [P, K17, width] with k stride NBW
                v = NB[:].rearrange("p (k j) -> p k j", j=NBW)
                return v[:, :, PH * t + off:PH * t + off + width]

            pos_c = nbv(8, t, 32, 3)        # [P, 3] center pos
            ori_c = nbv(8, t, 35, 9)        # [P, 9] center frame

            # ---- geometry -> dav [P, (k,8)] fp16 -------------------------
            D = wrk.tile([P, K17 * 3], f16, tag="D")
            Dv = D[:].rearrange("p (k a) -> p k a", a=3)
            nc.vector.tensor_sub(Dv, kview(32, 3),
                                 pos_c.unsqueeze(1).broadcast_to([P, K17, 3]))
            sq = wrk.tile([P, K17 * 3], f16, tag="sq")
            nc.vector.tensor_mul(sq[:], D[:], D[:])
            d2 = wrk.tile([P, K17], f32, tag="d2")
            nc.vector.tensor_reduce(d2[:], sq[:].rearrange("p (k a) -> p k a", a=3),
                                    axis=AX.X, op=OP.add)
            dav = wrk.tile([P, K17 * 8], f16, tag="dav")
            davv = dav[:].rearrange("p (k d) -> p k d", d=8)
            # dist/R into delta slot 6 (sqrt(d2)/R)
            nc.scalar.activation(davv[:, :, 6], d2[:], AF.Sqrt, bias=0.0,
                                 scale=1.0 / (R * R))
            dist = wrk.tile([P, K17], f32, tag="dist")
            nc.scalar.activation(dist[:], d2[:], AF.Sqrt, bias=eps_sb[0:P, 0:1],
                                 scale=1.0)
            rec = wrk.tile([P, K17], f16, tag="rec")
            with nc.allow_low_precision(reason="fp16 direction scale is ok"):
                nc.vector.reciprocal(rec[:], dist[:])
            # local_a = (sum_b Ri[a,b] * D[k,b]) * rec[k]
            lm = wrk.tile([P, K17 * 9], f16, tag="lm")
            lmv = lm[:].rearrange("p (k a b) -> p k a b", a=3, b=3)
            nc.vector.tensor_mul(
                lmv,
                ori_c.rearrange("p (a b) -> p a b", b=3).unsqueeze(1)
                     .broadcast_to([P, K17, 3, 3]),
                D[:].rearrange("p (k b) -> p k b", b=3).unsqueeze(2)
                    .broadcast_to([P, K17, 3, 3]))
            locr = wrk.tile([P, K17 * 3], f16, tag="locr")
            with nc.allow_low_precision(reason="3-term sums, fp16 ok"):
                nc.vector.tensor_reduce(
                    locr[:].rearrange("p (k a) -> p k a", a=3), lmv,
                    axis=AX.X, op=OP.add)
            nc.vector.tensor_mul(
                davv[:, :, 0:3], locr[:].rearrange("p (k a) -> p k a", a=3),
                rec[:].unsqueeze(-1).broadcast_to([P, K17, 3]))
            # ofeat_a = sum_b Ri[a,b] * Rj[a,b]
            ofm = wrk.tile([P, K17 * 9], f16, tag="ofm")
            nc.vector.tensor_mul(
                ofm[:].rearrange("p (k e) -> p k e", e=9), kview(35, 9),
                ori_c.unsqueeze(1).broadcast_to([P, K17, 9]))
            with nc.allow_low_precision(reason="3-term sums, fp16 ok"):
                nc.vector.tensor_reduce(
                    davv[:, :, 3:6],
                    ofm[:].rearrange("p (k a b) -> p k a b", a=3, b=3),
                    axis=AX.X, op=OP.add)
            nc.vector.memset(davv[:, :, 7], 1.0)
            # chain-boundary mask (zeroes whole delta rows incl. bias);
            # host-expanded to (k, 8) so all operands are packed (2x mode)
            nc.vector.tensor_mul(
                dav[:], dav[:],
                mask_sb[0:P, K17 * 8 * t:K17 * 8 * (t + 1)])

            # ---- kern2 = lrelu(dav @ WS2, 0.2), pair-duplicated ----------
            dT_p = psD.tile([128, 224], f16, tag="dT")
            nc.tensor.matmul(dT_p[:, 0:P], dav[:, 0:128], id_sb[0:P, 0:P],
                             is_transpose=True, start=True, stop=False,
                             skip_group_check=True)
            nc.tensor.matmul(dT_p[0:8, P:P + P], dav[:, 128:136], id_sb[0:P, 0:P],
                             is_transpose=True, start=False, stop=True,
                             skip_group_check=True)
            dT = wrk.tile([128, 224], f16, tag="dT_sb")
            nc.scalar.copy(dT[:], dT_p[:])
            W2 = 2 * K17 * KC  # 816
            # psum banks are 512 f32 cols: put k-blocks 0..9 at 0:480 (bank 0)
            # and k-blocks 10..16 at 512:848 (bank 1) to avoid bank crossing.
            pre_p = psP.tile([P, 848], f32, tag="pre")
            nc.tensor.matmul(pre_p[:, 0:480], dT[:, 0:P], ws2a_sb[:, 0:480],
                             start=True, stop=False, skip_group_check=True)
            nc.tensor.matmul(pre_p[:, 512:848], dT[:, 0:P], ws2a_sb[:, 480:W2],
                             start=True, stop=False, skip_group_check=True)
            nc.tensor.matmul(pre_p[:, 0:480], dT[0:8, P:P + P], ws2b_sb[:, 0:480],
                             start=False, stop=True, skip_group_check=True)
            nc.tensor.matmul(pre_p[:, 512:848], dT[0:8, P:P + P], ws2b_sb[:, 480:W2],
                             start=False, stop=True, skip_group_check=True)
            kern2 = wrk.tile([P, W2], f16, tag="kern2")
            nc.scalar.activation(kern2[:, 0:480], pre_p[:, 0:480], AF.Prelu,
                                 bias=0.0, scale=1.0, alpha=NEG_K)
            nc.scalar.activation(kern2[:, 480:W2], pre_p[:, 512:848], AF.Prelu,
                                 bias=0.0, scale=1.0, alpha=NEG_K)
            # self-edge compensation (host-precomputed, pair-duplicated)
            K8 = 2 * KC * 8
            nc.vector.tensor_add(kern2[:, K8:K8 + 2 * KC],
                                 kern2[:, K8:K8 + 2 * KC],
                                 ks2_sb[0:P, 2 * KC * t:2 * KC * (t + 1)])

            # ---- bilinear + PE transpose-accumulate ----------------------
            # gpsimd takes the last 3 offsets (issued first so they finish
            # by the time the PE transpose chain reaches them); DVE does the
            # rest in the 2x packed mode.
            # sum_k kern_k (x) h_k accumulated in normal layout via
            # identity-stationary copy-matmuls (2 per k, split at the psum
            # bank boundary), then ONE transpose set of 6 matmuls.
            agg_p = psG.tile([P, 768], f32, tag="agg")
            for k in range(K17):
                tm = tpool.tile([P, KC * W], f16, tag="tm")
                hv = nbv(k, t, 0, 32).rearrange("p (s two) -> p s two", two=2) \
                    .unsqueeze(1).broadcast_to([P, KC, 16, 2])
                kv = kern2[:, 2 * KC * k:2 * KC * (k + 1)] \
                    .rearrange("p (c two) -> p c two", two=2) \
                    .unsqueeze(2).broadcast_to([P, KC, 16, 2])
                nc.vector.tensor_tensor(
                    tm[:].rearrange("p (c s two) -> p c s two", two=2, s=16),
                    hv, kv, op=OP.mult)
                nc.tensor.matmul(agg_p[:, 0:512], id_sb[0:P, 0:P],
                                 tm[:, 0:512], start=(k == 0), stop=(k == 16),
                                 skip_group_check=True)
                nc.tensor.matmul(agg_p[:, 512:768], id_sb[0:P, 0:P],
                                 tm[:, 512:768], start=(k == 0), stop=(k == 16),
                                 skip_group_check=True)
            agg = wrk.tile([P, 768], f16, tag="agg_sb")
            nc.scalar.copy(agg[:], agg_p[:])
            aggT_p = psA.tile([128, 768], f32, tag="aggT")
            for b in range(6):
                nc.tensor.matmul(aggT_p[:, 128 * b:128 * b + P],
                                 agg[:, 128 * b:128 * (b + 1)], id_sb[0:P, 0:P],
                                 start=(b in (0, 4)), stop=(b in (3, 5)),
                                 skip_group_check=True)
            aggT = wrk.tile([128, 768], f16, tag="aggT_sb")
            nc.scalar.copy(aggT[:], aggT_p[:])

            # ---- conv = lrelu(agg @ Wk, 0.1) ; out = conv @ W_out + x ----
            co_p = psC.tile([P, 240], f32, tag="co")
            for b in range(6):
                nc.tensor.matmul(co_p[0:W, 0:P], wk_sb[:, W * b:W * (b + 1)],
                                 aggT[:, 128 * b:128 * b + P],
                                 start=(b == 0), stop=(b == 5),
                                 skip_group_check=True)
            convL = wrk.tile([W, P], f16, tag="convL")
            nc.scalar.activation(convL[:], co_p[0:W, 0:P], AF.Prelu, bias=0.0,
                                 scale=1.0, alpha=NEG_IN)
            nc.tensor.matmul(co_p[:, P:P + 128], convL[:], w_out_sb[:],
                             start=True, stop=False, skip_group_check=True)
            # identity add on the PE: accumulate xc into the same psum group
            # via an identity-stationary copy-matmul, then DMA from PSUM.
            nc.tensor.matmul(co_p[:, P:P + 128], id_sb[0:P, 0:P],
                             xc_all[0:P, C * t:C * t + C],
                             start=False, stop=True, skip_group_check=True)
            out_sb = wrk.tile([P, C], f32, tag="out_sb")
            nc.scalar.copy(out_sb[:], co_p[:, P:P + 128])
            cnt = min(TS, NPC - TS * t)
            nc.sync.dma_start(y[TS * t:TS * t + cnt, :], out_sb[0:cnt, :])

    nc.compile()
    return nc


def _expected_src_dst():
    i = np.arange(N)
    offs = np.arange(-WIN, WIN + 1)
    j = i[:, None] + offs[None, :]
    valid = ((j // L) == (i[:, None] // L)) & (j >= 0) & (j < N)
    j = np.where(valid, j, i[:, None])
    dst = np.repeat(i, offs.size).astype(np.int32)
    src = j.reshape(-1).astype(np.int32)
    return src, dst


def _host_inputs(x, pos, ori, W_in, Ws0, bs0, Wk, W_out):
    xf = np.ascontiguousarray(x.reshape(N, C), np.float32)
    pos = np.asarray(pos, np.float32)
    ori = np.asarray(ori, np.float32)
    f16 = np.float16

    # shared weights / constants
    WS = np.zeros((136, K17 * KC), np.float32)
    for k in range(K17):
        s = _sidx(k)
        WS[8 * k:8 * k + 7, KC * k:KC * (k + 1)] = Ws0[s]
        WS[8 * k + 7, KC * k:KC * (k + 1)] = bs0[s]
    # pair-duplicate columns: WS2[:, 48k + 2c + j] = WS[:, 24k + c]
    WS2 = np.repeat(WS, 2, axis=1)
    wk_p = np.zeros((128, 6 * W), np.float32)
    for b in range(6):
        wk_p[:, W * b:W * (b + 1)] = Wk[128 * b:128 * (b + 1), :]
    shifts = np.zeros((128, K17 * TS), np.float32)
    for k in range(K17):
        for p in range(TS):
            shifts[p + k, TS * k + p] = 1.0
    common = dict(
        w_in=W_in.astype(f16),
        ws2a=WS2[0:128].astype(f16),
        ws2b=WS2[128:136].astype(f16),
        wk_p=wk_p.astype(f16),
        w_out=W_out.astype(f16),
        ident=np.eye(128, dtype=f16),
        shifts=shifts.astype(f16),
    )

    # self-edge compensation: kself[n] = lrelu(rn @ W5[3:6] + b5, 0.2) * ncl
    rn = (ori.reshape(N, 3, 3) ** 2).sum(axis=2)          # [N, 3]
    pself = rn @ np.asarray(Ws0[S_HALF][3:6], np.float32) \
        + np.asarray(bs0[S_HALF], np.float32)             # [N, KC]
    kself_full = np.where(pself >= 0, pself, NEG_K * pself)

    in_maps = []
    for ci in range(NCORES):
        s0 = ci * NPC
        g = s0 - WIN + np.arange(HR)
        ok = (g >= 0) & (g < N)
        gi = np.clip(g, 0, N - 1)
        x_pad = np.where(ok[:, None], xf[gi], 0.0).astype(np.float32)
        p_pad = np.where(ok[:, None], pos[gi], 0.0).astype(np.float32)
        o_pad = np.where(ok[:, None], ori[gi], 0.0).astype(np.float32)

        jj, pp = np.meshgrid(np.arange(NT), np.arange(128), indexing="ij")
        rows = (TS * jj + pp)            # [NT, 128] all < HR
        # xT_slot: [128(c), (t, p)] transposed slots
        x_sl = x_pad[rows]               # [NT, 128, C]
        xT_slot = np.ascontiguousarray(
            x_sl.transpose(2, 0, 1).reshape(C, NT * 128)).astype(f16)
        # pos: center per slot for fp16 precision
        p_sl = p_pad[rows]               # [NT, 128, 3]
        ctr = p_sl.mean(axis=1, keepdims=True)
        p_sl = (p_sl - ctr).astype(f16)
        pos_slot = np.ascontiguousarray(
            p_sl.transpose(1, 0, 2).reshape(128, NT * 3))
        o_sl = o_pad[rows].astype(f16)
        ori_slot = np.ascontiguousarray(
            o_sl.transpose(1, 0, 2).reshape(128, NT * 9))
        # identity (center rows)
        rc = WIN + TS * jj + pp
        okc = rc < HR
        xc_slot = np.where(okc[:, :, None], x_pad[np.minimum(rc, HR - 1)], 0.0)
        xc_slot = xc_slot.transpose(1, 0, 2).reshape(128, NT * C).astype(f16)

        # mask + boundary-count + kself2 (output-node indexed)
        mask = np.zeros((128, NT, K17), np.float32)
        ncl = np.zeros((128, NT), np.float32)
        for t in range(NT):
            nvalid = min(TS, NPC - TS * t)
            for p in range(nvalid):
                n = s0 + TS * t + p
                off = n % L
                v = ((off + np.arange(-WIN, WIN + 1)) >= 0) & \
                    ((off + np.arange(-WIN, WIN + 1)) < L)
                mask[p, t, :] = v.astype(np.float32)
                ncl[p, t] = K17 - v.sum()
        ks = np.zeros((128, NT, KC), np.float32)
        for t in range(NT):
            nvalid = min(TS, NPC - TS * t)
            rowsn = s0 + TS * t + np.arange(nvalid)
            ks[:nvalid, t, :] = kself_full[rowsn] * ncl[:nvalid, t][:, None]
        ks2 = np.repeat(ks, 2, axis=2)  # duplicate pairs within each KC block
        in_maps.append(dict(
            xT_slot=xT_slot, xc_slot=xc_slot,
            pos_slot=pos_slot, ori_slot=ori_slot,
            maskd=np.repeat(mask.reshape(128, NT, K17, 1), 8, axis=3)
            .reshape(128, NT * K17 * 8).astype(f16),
            kself2=ks2.reshape(128, NT * 2 * KC).astype(f16),
            **common))
    return in_maps


def kernel(x, pos, seq, ori, W_in, Ws0, bs0, Wk, W_out, src, dst):
    exp_src, exp_dst = _expected_src_dst()
    assert np.array_equal(np.asarray(src), exp_src), "unexpected src graph"
    assert np.array_equal(np.asarray(dst), exp_dst), "unexpected dst graph"

    from concourse.bass_utils import run_bass_kernel_spmd

    if "nc" not in _PROG:
        _PROG["nc"] = _build_program()
    nc = _PROG["nc"]

    in_maps = _host_inputs(np.asarray(x), np.asarray(pos), np.asarray(ori),
                           np.asarray(W_in), np.asarray(Ws0), np.asarray(bs0),
                           np.asarray(Wk), np.asarray(W_out))
    res = run_bass_kernel_spmd(nc, in_maps, list(range(NCORES)))
    out = np.concatenate([res.results[i]["y"] for i in range(NCORES)], axis=0)
    return out.reshape(B, L, C).astype(np.float32)


# revision 45
# speedup vs baseline: 1.1299x; 1.0136x over previous
"""Bass/Trainium2 kernel for nn_BasicBlock_73933567033945 (CDConv / gnn_message_passing).

v2 strategy (graph = fixed +-8 sequence window inside 4 chains, verified at
runtime): shard 8192 nodes across 8 cores (1024 each, half a chain), slot
layout of 128-row halo windows at stride 112.  All matmuls and DVE tensor ops
run in fp16 (fp32 PSUM accumulation); pos is slot-centered on host so fp16
holds precision.  The 17 window shifts are materialized once per core by 17
wide shift-matmuls over all 10 slots (h|pos|ori, 440 cols each).  The
per-edge kernel MLP output is written pair-duplicated (kern2) so the
bilinear kern (x) h product runs in the DVE 2x perf mode.  The (offset,
channel) contraction runs on the PE via PSUM-accumulated transposes followed
by Wk-chunk matmuls, all fp16.  Pure data parallel: no collectives.
"""
import numpy as np

B, L, C = 4, 2048, 128
N = B * L
W = 32
KC = 24
SEQ_L = 11
R = 12.0
WIN = 8
NEG_IN = 0.1
NEG_K = 0.2
NCORES = 8
NPC = N // NCORES          # 1024 nodes per core
TS = 112                   # output nodes per tile
NT = 10                    # tiles per core (9*112 + 16)
HR = 9 * TS + 128          # 1136 halo rows per core
K17 = 2 * WIN + 1          # 17 window offsets
S_HALF = SEQ_L // 2
PH = 44                    # phys cols per slot: h(32) | pos(3) | ori(9)
NBW = NT * PH              # 440: NB cols per k

_PROG = {}


def _sidx(k):
    return int(np.clip(k - WIN, -S_HALF, S_HALF)) + S_HALF


def _build_program():
    import concourse.tile as tile
    from concourse import mybir, bacc
    from concourse.bass_utils import run_bass_kernel_spmd  # noqa: F401 (import check)
    from contextlib import ExitStack

    f32 = mybir.dt.float32
    f16 = mybir.dt.float16
    AF = mybir.ActivationFunctionType
    OP = mybir.AluOpType
    AX = mybir.AxisListType

    nc = bacc.Bacc("TRN2", target_bir_lowering=False, debug=False)

    def din(name, shape, dt=f16):
        return nc.dram_tensor(name, shape, dt, kind="ExternalInput").ap()

    xT_slot = din("xT_slot", [128, NT * 128])        # x transposed per slot, f16
    xc_slot = din("xc_slot", [128, NT * C])          # identity (center rows) f16
    pog_slot = din("pog_slot", [128, NT * 12])       # centered pos(3) | ori(9)
    w_in = din("w_in", [C, W])
    ws2a = din("ws2a", [128, 2 * K17 * KC])
    ws2b = din("ws2b", [8, 2 * K17 * KC])
    wk_p = din("wk_p", [128, 6 * W])
    w_out = din("w_out", [W, C])
    ident = din("ident", [128, 128])
    shifts = din("shifts", [128, K17 * TS])
    kself2 = din("kself2", [128, NT * 2 * KC])
    y = nc.dram_tensor("y", [NPC, C], f32, kind="ExternalOutput").ap()

    P = TS  # 112 active partitions

    with tile.TileContext(nc) as tc, ExitStack() as ctx:
        pers = ctx.enter_context(tc.tile_pool(name="pers", bufs=1))

        def load(ap_in, shape, tag, dt=f16):
            t = pers.tile(shape, dt, tag=tag)
            nc.sync.dma_start(t[:], ap_in)
            return t

        xT_all = load(xT_slot, [128, NT * 128], "xT_all")
        xc_all = load(xc_slot, [128, NT * C], "xc_all")
        w_in_sb = load(w_in, [C, W], "w_in")
        ws2a_sb = load(ws2a, [128, 2 * K17 * KC], "ws2a")
        ws2b_sb = load(ws2b, [8, 2 * K17 * KC], "ws2b")
        wk_sb = load(wk_p, [128, 6 * W], "wk")
        w_out_sb = load(w_out, [W, C], "w_out")
        id_sb = load(ident, [128, 128], "ident")
        sh_sb = load(shifts, [128, K17 * TS], "shifts")
        ks2_sb = load(kself2, [128, NT * 2 * KC], "kself2")

        # dist = sqrt(d2 + eps): eps = 1e-4 keeps rec = 1/dist <= 100 (fp16
        # safe; self-edges have D = 0 so local = 0 regardless) while real
        # edge distances (>= ~0.5) are perturbed by < 1e-3 relative.
        eps_sb = pers.tile([128, 1], f32, tag="eps")
        nc.vector.memset(eps_sb[:], 1e-4)

        phys_g = pers.tile([128, NT * 12], f16, tag="phys_g")
        nc.sync.dma_start(phys_g[:], pog_slot)
        phys_h = pers.tile([128, NT * W], f16, tag="phys_h")

        # ---- NBg: 17 pos/ori shift matmuls (independent of Phase A) ------
        NBg = pers.tile([P, K17 * NT * 12], f16, tag="NBg")
        NBh = pers.tile([P, K17 * NT * W], f16, tag="NBh")
        GW = NT * 12   # 120
        HW_ = NT * W   # 320
        with tc.tile_pool(name="pNBg", bufs=3, space="PSUM") as pNBg:
            for k in range(K17):
                nb_p = pNBg.tile([P, GW], f32, tag="nbg_p")
                nc.tensor.matmul(nb_p[:], sh_sb[:, TS * k:TS * (k + 1)],
                                 phys_g[:], start=True, stop=True)
                nc.vector.tensor_copy(NBg[:, GW * k:GW * (k + 1)], nb_p[:])

        # ---------------- Phase A: h = lrelu(lrelu(x) @ W_in) per slot -----
        with tc.tile_pool(name="pA", bufs=2) as pA, \
             tc.tile_pool(name="pAp", bufs=2, space="PSUM") as pAp:
            for j in range(NT):
                xlT = pA.tile([128, 128], f16, tag="xlT")
                nc.scalar.activation(xlT[:], xT_all[:, 128 * j:128 * (j + 1)],
                                     AF.Prelu, bias=0.0, scale=1.0, alpha=NEG_IN)
                hp = pAp.tile([128, W], f32, tag="hp")
                nc.tensor.matmul(hp[:], xlT[:], w_in_sb[:], start=True, stop=True)
                nc.scalar.activation(phys_h[:, W * j:W * (j + 1)], hp[:],
                                     AF.Prelu, bias=0.0, scale=1.0, alpha=NEG_IN)

        # ---- NBh: 17 h shift matmuls ------------------------------------
        with tc.tile_pool(name="pNBh", bufs=3, space="PSUM") as pNBh:
            for k in range(K17):
                nb_p = pNBh.tile([P, HW_], f32, tag="nbh_p")
                nc.tensor.matmul(nb_p[:], sh_sb[:, TS * k:TS * (k + 1)],
                                 phys_h[:], start=True, stop=True)
                dst = NBh[:, HW_ * k:HW_ * (k + 1)]
                if k % 2 == 0:
                    nc.scalar.copy(dst, nb_p[:])
                else:
                    nc.vector.tensor_copy(dst, nb_p[:])

        # ---------------- Phase B: per output tile ------------------------
        wrk = ctx.enter_context(tc.tile_pool(name="wrk", bufs=3))
        tpool = ctx.enter_context(tc.tile_pool(name="tmp", bufs=6))
        psG = ctx.enter_context(tc.tile_pool(name="psG", bufs=1, space="PSUM"))
        psA = ctx.enter_context(tc.tile_pool(name="psA", bufs=1, space="PSUM"))
        psD = ctx.enter_context(tc.tile_pool(name="psD", bufs=1, space="PSUM"))
        psP = ctx.enter_context(tc.tile_pool(name="psP", bufs=1, space="PSUM"))
        psC = ctx.enter_context(tc.tile_pool(name="psC", bufs=1, space="PSUM"))

        for t in range(NT):
            # k-strided views into NBg for slot t (pos at off 0, ori at 3)
            def kview(off, width):
                v = NBg[:].rearrange("p (k j) -> p k j", j=GW)
                return v[:, :, 12 * t + off:12 * t + off + width]

            gc = GW * 8 + 12 * t
            pos_c = NBg[:, gc:gc + 3]       # [P, 3] center pos
            ori_c = NBg[:, gc + 3:gc + 12]  # [P, 9] center frame

            # ---- geometry -> dav [P, (k,8)] fp16 -------------------------
            D = wrk.tile([P, K17 * 3], f16, tag="D")
            Dv = D[:].rearrange("p (k a) -> p k a", a=3)
            nc.vector.tensor_sub(Dv, kview(0, 3),
                                 pos_c.unsqueeze(1).broadcast_to([P, K17, 3]))
            sq = wrk.tile([P, K17 * 3], f16, tag="sq")
            nc.vector.tensor_mul(sq[:], D[:], D[:])
            d2 = wrk.tile([P, K17], f32, tag="d2")
            nc.vector.tensor_reduce(d2[:], sq[:].rearrange("p (k a) -> p k a", a=3),
                                    axis=AX.X, op=OP.add)
            dav = wrk.tile([P, K17 * 8], f16, tag="dav")
            davv = dav[:].rearrange("p (k d) -> p k d", d=8)
            # dist/R into delta slot 6 (sqrt(d2)/R)
            nc.scalar.activation(davv[:, :, 6], d2[:], AF.Sqrt, bias=0.0,
                                 scale=1.0 / (R * R))
            dist = wrk.tile([P, K17], f32, tag="dist")
            nc.scalar.activation(dist[:], d2[:], AF.Sqrt, bias=eps_sb[0:P, 0:1],
                                 scale=1.0)
            rec = wrk.tile([P, K17], f16, tag="rec")
            with nc.allow_low_precision(reason="fp16 direction scale is ok"):
                nc.vector.reciprocal(rec[:], dist[:])
            # local_a = (sum_b Ri[a,b] * D[k,b]) * rec[k]
            lm = wrk.tile([P, K17 * 9], f16, tag="lm")
            lmv = lm[:].rearrange("p (k a b) -> p k a b", a=3, b=3)
            nc.vector.tensor_mul(
                lmv,
                ori_c.rearrange("p (a b) -> p a b", b=3).unsqueeze(1)
                     .broadcast_to([P, K17, 3, 3]),
                D[:].rearrange("p (k b) -> p k b", b=3).unsqueeze(2)
                    .broadcast_to([P, K17, 3, 3]))
            locr = wrk.tile([P, K17 * 3], f16, tag="locr")
            with nc.allow_low_precision(reason="3-term sums, fp16 ok"):
                nc.vector.tensor_reduce(
                    locr[:].rearrange("p (k a) -> p k a", a=3), lmv,
                    axis=AX.X, op=OP.add)
            nc.vector.tensor_mul(
                davv[:, :, 0:3], locr[:].rearrange("p (k a) -> p k a", a=3),
                rec[:].unsqueeze(-1).broadcast_to([P, K17, 3]))
            # ofeat_a = sum_b Ri[a,b] * Rj[a,b]
            ofm = wrk.tile([P, K17 * 9], f16, tag="ofm")
            nc.vector.tensor_mul(
                ofm[:].rearrange("p (k e) -> p k e", e=9), kview(3, 9),
                ori_c.unsqueeze(1).broadcast_to([P, K17, 9]))
            with nc.allow_low_precision(reason="3-term sums, fp16 ok"):
                nc.vector.tensor_reduce(
                    davv[:, :, 3:6],
                    ofm[:].rearrange("p (k a b) -> p k a b", a=3, b=3),
                    axis=AX.X, op=OP.add)
            # out-of-chain halo rows are zero-padded on host, so h_src = 0
            # there and fake-edge messages vanish without any masking.  The
            # dav bias slot (7) is only ever written here, so set it just
            # once per pool buffer rotation.
            if t < 3:
                nc.vector.memset(davv[:, :, 7], 1.0)

            # ---- kern2 = lrelu(dav @ WS2, 0.2), pair-duplicated ----------
            dT_p = psD.tile([128, 224], f16, tag="dT")
            nc.tensor.matmul(dT_p[:, 0:P], dav[:, 0:128], id_sb[0:P, 0:P],
                             is_transpose=True, start=True, stop=False,
                             skip_group_check=True)
            nc.tensor.matmul(dT_p[0:8, P:P + P], dav[:, 128:136], id_sb[0:P, 0:P],
                             is_transpose=True, start=False, stop=True,
                             skip_group_check=True)
            dT = wrk.tile([128, 224], f16, tag="dT_sb")
            nc.scalar.copy(dT[:], dT_p[:])
            W2 = 2 * K17 * KC  # 816
            # psum banks are 512 f32 cols: put k-blocks 0..9 at 0:480 (bank 0)
            # and k-blocks 10..16 at 512:848 (bank 1) to avoid bank crossing.
            pre_p = psP.tile([P, 848], f32, tag="pre")
            nc.tensor.matmul(pre_p[:, 0:480], dT[:, 0:P], ws2a_sb[:, 0:480],
                             start=True, stop=False, skip_group_check=True)
            nc.tensor.matmul(pre_p[:, 512:848], dT[:, 0:P], ws2a_sb[:, 480:W2],
                             start=True, stop=False, skip_group_check=True)
            nc.tensor.matmul(pre_p[:, 0:480], dT[0:8, P:P + P], ws2b_sb[:, 0:480],
                             start=False, stop=True, skip_group_check=True)
            nc.tensor.matmul(pre_p[:, 512:848], dT[0:8, P:P + P], ws2b_sb[:, 480:W2],
                             start=False, stop=True, skip_group_check=True)
            kern2 = wrk.tile([P, W2], f16, tag="kern2")
            nc.scalar.activation(kern2[:, 0:480], pre_p[:, 0:480], AF.Prelu,
                                 bias=0.0, scale=1.0, alpha=NEG_K)
            nc.scalar.activation(kern2[:, 480:W2], pre_p[:, 512:848], AF.Prelu,
                                 bias=0.0, scale=1.0, alpha=NEG_K)
            # self-edge compensation (host-precomputed, pair-duplicated);
            # nonzero only at chain ends, which land in tiles 0 and 9
            if t in (0, NT - 1):
                K8 = 2 * KC * 8
                nc.vector.tensor_add(kern2[:, K8:K8 + 2 * KC],
                                     kern2[:, K8:K8 + 2 * KC],
                                     ks2_sb[0:P, 2 * KC * t:2 * KC * (t + 1)])

            # ---- bilinear + PE transpose-accumulate ----------------------
            # gpsimd takes the last 3 offsets (issued first so they finish
            # by the time the PE transpose chain reaches them); DVE does the
            # rest in the 2x packed mode.
            # sum_k kern_k (x) h_k accumulated in normal layout via
            # identity-stationary copy-matmuls (2 per k, split at the psum
            # bank boundary), then ONE transpose set of 6 matmuls.
            agg_p = psG.tile([P, 768], f32, tag="agg")
            for k in range(K17):
                tm = tpool.tile([P, KC * W], f16, tag="tm")
                hv = NBh[:, HW_ * k + W * t:HW_ * k + W * (t + 1)] \
                    .rearrange("p (s two) -> p s two", two=2) \
                    .unsqueeze(1).broadcast_to([P, KC, 16, 2])
                kv = kern2[:, 2 * KC * k:2 * KC * (k + 1)] \
                    .rearrange("p (c two) -> p c two", two=2) \
                    .unsqueeze(2).broadcast_to([P, KC, 16, 2])
                nc.vector.tensor_tensor(
                    tm[:].rearrange("p (c s two) -> p c s two", two=2, s=16),
                    hv, kv, op=OP.mult)
                nc.tensor.matmul(agg_p[:, 0:512], id_sb[0:P, 0:P],
                                 tm[:, 0:512], start=(k == 0), stop=(k == 16),
                                 skip_group_check=True)
                nc.tensor.matmul(agg_p[:, 512:768], id_sb[0:P, 0:P],
                                 tm[:, 512:768], start=(k == 0), stop=(k == 16),
                                 skip_group_check=True)
            agg = wrk.tile([P, 768], f16, tag="agg_sb")
            nc.scalar.copy(agg[:], agg_p[:])
            aggT_p = psA.tile([128, 768], f32, tag="aggT")
            for b in range(6):
                nc.tensor.matmul(aggT_p[:, 128 * b:128 * b + P],
                                 agg[:, 128 * b:128 * (b + 1)], id_sb[0:P, 0:P],
                                 start=(b in (0, 4)), stop=(b in (3, 5)),
                                 skip_group_check=True)
            aggT = wrk.tile([128, 768], f16, tag="aggT_sb")
            nc.scalar.copy(aggT[:], aggT_p[:])

            # ---- conv = lrelu(agg @ Wk, 0.1) ; out = conv @ W_out + x ----
            co_p = psC.tile([P, 240], f32, tag="co")
            for b in range(6):
                nc.tensor.matmul(co_p[0:W, 0:P], wk_sb[:, W * b:W * (b + 1)],
                                 aggT[:, 128 * b:128 * b + P],
                                 start=(b == 0), stop=(b == 5),
                                 skip_group_check=True)
            convL = wrk.tile([W, P], f16, tag="convL")
            nc.scalar.activation(convL[:], co_p[0:W, 0:P], AF.Prelu, bias=0.0,
                                 scale=1.0, alpha=NEG_IN)
            nc.tensor.matmul(co_p[:, P:P + 128], convL[:], w_out_sb[:],
                             start=True, stop=False, skip_group_check=True)
            # identity add on the PE: accumulate xc into the same psum group
            # via an identity-stationary copy-matmul, then DMA from PSUM.
            nc.tensor.matmul(co_p[:, P:P + 128], id_sb[0:P, 0:P],
                             xc_all[0:P, C * t:C * t + C],
                             start=False, stop=True, skip_group_check=True)
            out_sb = wrk.tile([P, C], f32, tag="out_sb")
            nc.scalar.copy(out_sb[:], co_p[:, P:P + 128])
            cnt = min(TS, NPC - TS * t)
            nc.sync.dma_start(y[TS * t:TS * t + cnt, :], out_sb[0:cnt, :])

    nc.compile()
    return nc


def _expected_src_dst():
    i = np.arange(N)
    offs = np.arange(-WIN, WIN + 1)
    j = i[:, None] + offs[None, :]
    valid = ((j // L) == (i[:, None] // L)) & (j >= 0) & (j < N)
    j = np.where(valid, j, i[:, None])
    dst = np.repeat(i, offs.size).astype(np.int32)
    src = j.reshape(-1).astype(np.int32)
    return src, dst


def _host_inputs(x, pos, ori, W_in, Ws0, bs0, Wk, W_out):
    xf = np.ascontiguousarray(x.reshape(N, C), np.float32)
    pos = np.asarray(pos, np.float32)
    ori = np.asarray(ori, np.float32)
    f16 = np.float16

    # shared weights / constants
    WS = np.zeros((136, K17 * KC), np.float32)
    for k in range(K17):
        s = _sidx(k)
        WS[8 * k:8 * k + 7, KC * k:KC * (k + 1)] = Ws0[s]
        WS[8 * k + 7, KC * k:KC * (k + 1)] = bs0[s]
    # pair-duplicate columns: WS2[:, 48k + 2c + j] = WS[:, 24k + c]
    WS2 = np.repeat(WS, 2, axis=1)
    wk_p = np.zeros((128, 6 * W), np.float32)
    for b in range(6):
        wk_p[:, W * b:W * (b + 1)] = Wk[128 * b:128 * (b + 1), :]
    shifts = np.zeros((128, K17 * TS), np.float32)
    for k in range(K17):
        for p in range(TS):
            shifts[p + k, TS * k + p] = 1.0
    common = dict(
        w_in=W_in.astype(f16),
        ws2a=WS2[0:128].astype(f16),
        ws2b=WS2[128:136].astype(f16),
        wk_p=wk_p.astype(f16),
        w_out=W_out.astype(f16),
        ident=np.eye(128, dtype=f16),
        shifts=shifts.astype(f16),
    )

    # self-edge compensation: kself[n] = lrelu(rn @ W5[3:6] + b5, 0.2) * ncl
    rn = (ori.reshape(N, 3, 3) ** 2).sum(axis=2)          # [N, 3]
    pself = rn @ np.asarray(Ws0[S_HALF][3:6], np.float32) \
        + np.asarray(bs0[S_HALF], np.float32)             # [N, KC]
    kself_full = np.where(pself >= 0, pself, NEG_K * pself)

    in_maps = []
    for ci in range(NCORES):
        s0 = ci * NPC
        g = s0 - WIN + np.arange(HR)
        # chain-aware zero padding: out-of-chain halo rows get h = 0, so
        # their messages vanish with no explicit masking on device.
        ok = (g // L) == (s0 // L)
        gi = np.clip(g, 0, N - 1)
        x_pad = np.where(ok[:, None], xf[gi], 0.0).astype(np.float32)
        p_pad = np.where(ok[:, None], pos[gi], 0.0).astype(np.float32)
        o_pad = np.where(ok[:, None], ori[gi], 0.0).astype(np.float32)

        jj, pp = np.meshgrid(np.arange(NT), np.arange(128), indexing="ij")
        rows = (TS * jj + pp)            # [NT, 128] all < HR
        # xT_slot: [128(c), (t, p)] transposed slots
        x_sl = x_pad[rows]               # [NT, 128, C]
        xT_slot = np.ascontiguousarray(
            x_sl.transpose(2, 0, 1).reshape(C, NT * 128)).astype(f16)
        # pos: center per slot for fp16 precision; interleave with ori
        p_sl = p_pad[rows]               # [NT, 128, 3]
        ctr = p_sl.mean(axis=1, keepdims=True)
        pog = np.concatenate([p_sl - ctr, o_pad[rows]], axis=2)  # [NT,128,12]
        pog_slot = np.ascontiguousarray(
            pog.transpose(1, 0, 2).reshape(128, NT * 12)).astype(f16)
        # identity (center rows)
        rc = WIN + TS * jj + pp
        okc = rc < HR
        xc_slot = np.where(okc[:, :, None], x_pad[np.minimum(rc, HR - 1)], 0.0)
        xc_slot = xc_slot.transpose(1, 0, 2).reshape(128, NT * C).astype(f16)

        # boundary-count + kself2 (output-node indexed)
        ncl = np.zeros((128, NT), np.float32)
        for t in (0, NT - 1):
            nvalid = min(TS, NPC - TS * t)
            for p in range(nvalid):
                off = (s0 + TS * t + p) % L
                v = ((off + np.arange(-WIN, WIN + 1)) >= 0) & \
                    ((off + np.arange(-WIN, WIN + 1)) < L)
                ncl[p, t] = K17 - v.sum()
        ks = np.zeros((128, NT, KC), np.float32)
        for t in (0, NT - 1):
            nvalid = min(TS, NPC - TS * t)
            rowsn = s0 + TS * t + np.arange(nvalid)
            ks[:nvalid, t, :] = kself_full[rowsn] * ncl[:nvalid, t][:, None]
        ks2 = np.repeat(ks, 2, axis=2)  # duplicate pairs within each KC block
        in_maps.append(dict(
            xT_slot=xT_slot, xc_slot=xc_slot, pog_slot=pog_slot,
            kself2=ks2.reshape(128, NT * 2 * KC).astype(f16),
            **common))
    return in_maps


def kernel(x, pos, seq, ori, W_in, Ws0, bs0, Wk, W_out, src, dst):
    exp_src, exp_dst = _expected_src_dst()
    assert np.array_equal(np.asarray(src), exp_src), "unexpected src graph"
    assert np.array_equal(np.asarray(dst), exp_dst), "unexpected dst graph"

    from concourse.bass_utils import run_bass_kernel_spmd

    if "nc" not in _PROG:
        _PROG["nc"] = _build_program()
    nc = _PROG["nc"]

    in_maps = _host_inputs(np.asarray(x), np.asarray(pos), np.asarray(ori),
                           np.asarray(W_in), np.asarray(Ws0), np.asarray(bs0),
                           np.asarray(Wk), np.asarray(W_out))
    res = run_bass_kernel_spmd(nc, in_maps, list(range(NCORES)))
    out = np.concatenate([res.results[i]["y"] for i in range(NCORES)], axis=0)
    return out.reshape(B, L, C).astype(np.float32)


# revision 54
# speedup vs baseline: 1.1998x; 1.0618x over previous
"""Bass/Trainium2 kernel for nn_BasicBlock_73933567033945 (CDConv / gnn_message_passing).

v2 strategy (graph = fixed +-8 sequence window inside 4 chains, verified at
runtime): shard 8192 nodes across 8 cores (1024 each, half a chain), slot
layout of 128-row halo windows at stride 112.  All matmuls and DVE tensor ops
run in fp16 (fp32 PSUM accumulation); pos is slot-centered on host so fp16
holds precision.  The 17 window shifts are materialized once per core by 17
wide shift-matmuls over all 10 slots (h|pos|ori, 440 cols each).  The
per-edge kernel MLP output is written pair-duplicated (kern2) so the
bilinear kern (x) h product runs in the DVE 2x perf mode.  The (offset,
channel) contraction runs on the PE via PSUM-accumulated transposes followed
by Wk-chunk matmuls, all fp16.  Pure data parallel: no collectives.
"""
import numpy as np

B, L, C = 4, 2048, 128
N = B * L
W = 32
KC = 24
SEQ_L = 11
R = 12.0
WIN = 8
NEG_IN = 0.1
NEG_K = 0.2
NCORES = 8
NPC = N // NCORES          # 1024 nodes per core
TS = 112                   # output nodes per tile
NT = 10                    # tiles per core (9*112 + 16)
HR = 9 * TS + 128          # 1136 halo rows per core
K17 = 2 * WIN + 1          # 17 window offsets
S_HALF = SEQ_L // 2
PH = 44                    # phys cols per slot: h(32) | pos(3) | ori(9)
NBW = NT * PH              # 440: NB cols per k

_PROG = {}


def _sidx(k):
    return int(np.clip(k - WIN, -S_HALF, S_HALF)) + S_HALF


def _build_program():
    import concourse.tile as tile
    from concourse import mybir, bacc
    from concourse.bass_utils import run_bass_kernel_spmd  # noqa: F401 (import check)
    from contextlib import ExitStack

    f32 = mybir.dt.float32
    f16 = mybir.dt.float16
    AF = mybir.ActivationFunctionType
    OP = mybir.AluOpType
    AX = mybir.AxisListType

    nc = bacc.Bacc("TRN2", target_bir_lowering=False, debug=False)

    def din(name, shape, dt=f16):
        return nc.dram_tensor(name, shape, dt, kind="ExternalInput").ap()

    xT_slot = din("xT_slot", [128, NT * 128])        # x transposed per slot, f16
    xc_slot = din("xc_slot", [128, NT * C])          # identity (center rows) f16
    pog_slot = din("pog_slot", [128, NT * 12])       # centered pos(3) | ori(9)
    w_in = din("w_in", [C, W])
    ws2a = din("ws2a", [128, 2 * K17 * KC])
    ws2b = din("ws2b", [8, 2 * K17 * KC])
    wk_p = din("wk_p", [128, 6 * W])
    w_out = din("w_out", [W, C])
    ident = din("ident", [128, 128])
    shifts = din("shifts", [128, K17 * TS])
    kself2 = din("kself2", [128, NT * 2 * KC])
    y = nc.dram_tensor("y", [NPC, C], f32, kind="ExternalOutput").ap()

    P = TS  # 112 active partitions

    with tile.TileContext(nc) as tc, ExitStack() as ctx:
        pers = ctx.enter_context(tc.tile_pool(name="pers", bufs=1))

        def load(ap_in, shape, tag, dt=f16):
            t = pers.tile(shape, dt, tag=tag)
            nc.sync.dma_start(t[:], ap_in)
            return t

        # DMA order matters: pos/ori + shift matrices + identity feed the
        # NBg matmuls that start first; xc is only needed at tile ends.
        phys_g = pers.tile([128, NT * 12], f16, tag="phys_g")
        nc.sync.dma_start(phys_g[:], pog_slot)
        sh_sb = load(shifts, [128, K17 * TS], "shifts")
        id_sb = load(ident, [128, 128], "ident")
        w_in_sb = load(w_in, [C, W], "w_in")
        xT_all = pers.tile([128, NT * 128], f16, tag="xT_all")
        nc.sync.dma_start(xT_all[:, 0:5 * 128], xT_slot[:, 0:5 * 128])
        nc.sync.dma_start(xT_all[:, 5 * 128:], xT_slot[:, 5 * 128:])
        ws2a_sb = load(ws2a, [128, 2 * K17 * KC], "ws2a")
        ws2b_sb = load(ws2b, [8, 2 * K17 * KC], "ws2b")
        wk_sb = load(wk_p, [128, 6 * W], "wk")
        w_out_sb = load(w_out, [W, C], "w_out")
        ks2_sb = load(kself2, [128, NT * 2 * KC], "kself2")
        xc_all = load(xc_slot, [128, NT * C], "xc_all")

        # dist = sqrt(d2 + eps): eps = 1e-4 keeps rec = 1/dist <= 100 (fp16
        # safe; self-edges have D = 0 so local = 0 regardless) while real
        # edge distances (>= ~0.5) are perturbed by < 1e-3 relative.
        eps_sb = pers.tile([128, 1], f32, tag="eps")
        nc.vector.memset(eps_sb[:], 1e-4)
        phys_h = pers.tile([128, NT * W], f16, tag="phys_h")

        # ---- NBg: 17 pos/ori shift matmuls (independent of Phase A) ------
        NBg = pers.tile([P, K17 * NT * 12], f16, tag="NBg")
        NBh = pers.tile([P, K17 * NT * W], f16, tag="NBh")
        GW = NT * 12   # 120
        HW_ = NT * W   # 320
        with tc.tile_pool(name="pNBg", bufs=3, space="PSUM") as pNBg:
            for k in range(K17):
                nb_p = pNBg.tile([P, GW], f32, tag="nbg_p")
                nc.tensor.matmul(nb_p[:], sh_sb[:, TS * k:TS * (k + 1)],
                                 phys_g[:], start=True, stop=True)
                nc.vector.tensor_copy(NBg[:, GW * k:GW * (k + 1)], nb_p[:])

        # ---------------- Phase A: h = lrelu(lrelu(x) @ W_in) per slot -----
        with tc.tile_pool(name="pA", bufs=2) as pA, \
             tc.tile_pool(name="pAp", bufs=2, space="PSUM") as pAp:
            for j in range(NT):
                xlT = pA.tile([128, 128], f16, tag="xlT")
                nc.scalar.activation(xlT[:], xT_all[:, 128 * j:128 * (j + 1)],
                                     AF.Prelu, bias=0.0, scale=1.0, alpha=NEG_IN)
                hp = pAp.tile([128, W], f32, tag="hp")
                nc.tensor.matmul(hp[:], xlT[:], w_in_sb[:], start=True, stop=True)
                nc.scalar.activation(phys_h[:, W * j:W * (j + 1)], hp[:],
                                     AF.Prelu, bias=0.0, scale=1.0, alpha=NEG_IN)

        # ---- NBh: 17 h shift matmuls ------------------------------------
        with tc.tile_pool(name="pNBh", bufs=3, space="PSUM") as pNBh:
            for k in range(K17):
                nb_p = pNBh.tile([P, HW_], f32, tag="nbh_p")
                nc.tensor.matmul(nb_p[:], sh_sb[:, TS * k:TS * (k + 1)],
                                 phys_h[:], start=True, stop=True)
                dst = NBh[:, HW_ * k:HW_ * (k + 1)]
                if k % 2 == 0:
                    nc.scalar.copy(dst, nb_p[:])
                else:
                    nc.vector.tensor_copy(dst, nb_p[:])

        # ---------------- Phase B: per output tile ------------------------
        wrk = ctx.enter_context(tc.tile_pool(name="wrk", bufs=3))
        tpool = ctx.enter_context(tc.tile_pool(name="tmp", bufs=6))
        psG = ctx.enter_context(tc.tile_pool(name="psG", bufs=2, space="PSUM"))
        psD = ctx.enter_context(tc.tile_pool(name="psD", bufs=1, space="PSUM"))
        psP = ctx.enter_context(tc.tile_pool(name="psP", bufs=1, space="PSUM"))
        psC = ctx.enter_context(tc.tile_pool(name="psC", bufs=1, space="PSUM"))

        for t in range(NT):
            # k-strided views into NBg for slot t (pos at off 0, ori at 3)
            def kview(off, width):
                v = NBg[:].rearrange("p (k j) -> p k j", j=GW)
                return v[:, :, 12 * t + off:12 * t + off + width]

            gc = GW * 8 + 12 * t
            pos_c = NBg[:, gc:gc + 3]       # [P, 3] center pos
            ori_c = NBg[:, gc + 3:gc + 12]  # [P, 9] center frame

            # ---- geometry -> dav [P, (k,8)] fp16 -------------------------
            D = wrk.tile([P, K17 * 3], f16, tag="D")
            Dv = D[:].rearrange("p (k a) -> p k a", a=3)
            nc.vector.tensor_sub(Dv, kview(0, 3),
                                 pos_c.unsqueeze(1).broadcast_to([P, K17, 3]))
            sq = wrk.tile([P, K17 * 3], f16, tag="sq")
            nc.vector.tensor_mul(sq[:], D[:], D[:])
            d2 = wrk.tile([P, K17], f32, tag="d2")
            nc.vector.tensor_reduce(d2[:], sq[:].rearrange("p (k a) -> p k a", a=3),
                                    axis=AX.X, op=OP.add)
            dav = wrk.tile([P, K17 * 8], f16, tag="dav")
            davv = dav[:].rearrange("p (k d) -> p k d", d=8)
            # dist/R into delta slot 6 (sqrt(d2)/R)
            nc.scalar.activation(davv[:, :, 6], d2[:], AF.Sqrt, bias=0.0,
                                 scale=1.0 / (R * R))
            dist = wrk.tile([P, K17], f32, tag="dist")
            nc.scalar.activation(dist[:], d2[:], AF.Sqrt, bias=eps_sb[0:P, 0:1],
                                 scale=1.0)
            rec = wrk.tile([P, K17], f16, tag="rec")
            with nc.allow_low_precision(reason="fp16 direction scale is ok"):
                nc.vector.reciprocal(rec[:], dist[:])
            # local_a = (sum_b Ri[a,b] * D[k,b]) * rec[k]
            lm = wrk.tile([P, K17 * 9], f16, tag="lm")
            lmv = lm[:].rearrange("p (k a b) -> p k a b", a=3, b=3)
            nc.vector.tensor_mul(
                lmv,
                ori_c.rearrange("p (a b) -> p a b", b=3).unsqueeze(1)
                     .broadcast_to([P, K17, 3, 3]),
                D[:].rearrange("p (k b) -> p k b", b=3).unsqueeze(2)
                    .broadcast_to([P, K17, 3, 3]))
            locr = wrk.tile([P, K17 * 3], f16, tag="locr")
            with nc.allow_low_precision(reason="3-term sums, fp16 ok"):
                nc.vector.tensor_reduce(
                    locr[:].rearrange("p (k a) -> p k a", a=3), lmv,
                    axis=AX.X, op=OP.add)
            nc.vector.tensor_mul(
                davv[:, :, 0:3], locr[:].rearrange("p (k a) -> p k a", a=3),
                rec[:].unsqueeze(-1).broadcast_to([P, K17, 3]))
            # ofeat_a = sum_b Ri[a,b] * Rj[a,b]
            ofm = wrk.tile([P, K17 * 9], f16, tag="ofm")
            nc.vector.tensor_mul(
                ofm[:].rearrange("p (k e) -> p k e", e=9), kview(3, 9),
                ori_c.unsqueeze(1).broadcast_to([P, K17, 9]))
            with nc.allow_low_precision(reason="3-term sums, fp16 ok"):
                nc.vector.tensor_reduce(
                    davv[:, :, 3:6],
                    ofm[:].rearrange("p (k a b) -> p k a b", a=3, b=3),
                    axis=AX.X, op=OP.add)
            # out-of-chain halo rows are zero-padded on host, so h_src = 0
            # there and fake-edge messages vanish without any masking.  The
            # dav bias slot (7) is only ever written here, so set it just
            # once per pool buffer rotation.
            if t < 3:
                nc.vector.memset(davv[:, :, 7], 1.0)

            # ---- kern2 = lrelu(dav @ WS2, 0.2), pair-duplicated ----------
            dT_p = psD.tile([128, 224], f16, tag="dT")
            nc.tensor.matmul(dT_p[:, 0:P], dav[:, 0:128], id_sb[0:P, 0:P],
                             is_transpose=True, start=True, stop=False,
                             skip_group_check=True)
            nc.tensor.matmul(dT_p[0:8, P:P + P], dav[:, 128:136], id_sb[0:P, 0:P],
                             is_transpose=True, start=False, stop=True,
                             skip_group_check=True)
            dT = wrk.tile([128, 224], f16, tag="dT_sb")
            nc.scalar.copy(dT[:], dT_p[:])
            W2 = 2 * K17 * KC  # 816
            # psum banks are 512 f32 cols: put k-blocks 0..9 at 0:480 (bank 0)
            # and k-blocks 10..16 at 512:848 (bank 1) to avoid bank crossing.
            pre_p = psP.tile([P, 848], f32, tag="pre")
            nc.tensor.matmul(pre_p[:, 0:480], dT[:, 0:P], ws2a_sb[:, 0:480],
                             start=True, stop=False, skip_group_check=True)
            nc.tensor.matmul(pre_p[:, 512:848], dT[:, 0:P], ws2a_sb[:, 480:W2],
                             start=True, stop=False, skip_group_check=True)
            nc.tensor.matmul(pre_p[:, 0:480], dT[0:8, P:P + P], ws2b_sb[:, 0:480],
                             start=False, stop=True, skip_group_check=True)
            nc.tensor.matmul(pre_p[:, 512:848], dT[0:8, P:P + P], ws2b_sb[:, 480:W2],
                             start=False, stop=True, skip_group_check=True)
            kern2 = wrk.tile([P, W2], f16, tag="kern2")
            nc.scalar.activation(kern2[:, 0:480], pre_p[:, 0:480], AF.Prelu,
                                 bias=0.0, scale=1.0, alpha=NEG_K)
            nc.scalar.activation(kern2[:, 480:W2], pre_p[:, 512:848], AF.Prelu,
                                 bias=0.0, scale=1.0, alpha=NEG_K)
            # self-edge compensation (host-precomputed, pair-duplicated);
            # nonzero only at chain ends, which land in tiles 0 and 9
            if t in (0, NT - 1):
                K8 = 2 * KC * 8
                nc.vector.tensor_add(kern2[:, K8:K8 + 2 * KC],
                                     kern2[:, K8:K8 + 2 * KC],
                                     ks2_sb[0:P, 2 * KC * t:2 * KC * (t + 1)])

            # ---- bilinear + PE transpose-accumulate ----------------------
            # gpsimd takes the last 3 offsets (issued first so they finish
            # by the time the PE transpose chain reaches them); DVE does the
            # rest in the 2x packed mode.
            # sum_k kern_k (x) h_k accumulated in normal layout via
            # identity-stationary copy-matmuls (2 per k, split at the psum
            # bank boundary), then ONE transpose set of 6 matmuls.  The
            # same psum tile is reused for the transposed result after the
            # accumulation has been copied out (WAR dep keeps it safe).
            ag_ps = psG.tile([128, 768], f32, tag="agg")
            agg_p = ag_ps[0:P, :]
            for k in range(K17):
                tm = tpool.tile([P, KC * W], f16, tag="tm")
                hv = NBh[:, HW_ * k + W * t:HW_ * k + W * (t + 1)] \
                    .rearrange("p (s two) -> p s two", two=2) \
                    .unsqueeze(1).broadcast_to([P, KC, 16, 2])
                kv = kern2[:, 2 * KC * k:2 * KC * (k + 1)] \
                    .rearrange("p (c two) -> p c two", two=2) \
                    .unsqueeze(2).broadcast_to([P, KC, 16, 2])
                nc.vector.tensor_tensor(
                    tm[:].rearrange("p (c s two) -> p c s two", two=2, s=16),
                    hv, kv, op=OP.mult)
                nc.tensor.matmul(agg_p[:, 0:512], id_sb[0:P, 0:P],
                                 tm[:, 0:512], start=(k == 0), stop=(k == 16),
                                 skip_group_check=True)
                nc.tensor.matmul(agg_p[:, 512:768], id_sb[0:P, 0:P],
                                 tm[:, 512:768], start=(k == 0), stop=(k == 16),
                                 skip_group_check=True)
            agg = wrk.tile([P, 768], f16, tag="agg_sb")
            nc.scalar.copy(agg[:], agg_p[:])
            aggT_p = ag_ps
            for b in range(6):
                nc.tensor.matmul(aggT_p[:, 128 * b:128 * b + P],
                                 agg[:, 128 * b:128 * (b + 1)], id_sb[0:P, 0:P],
                                 start=(b in (0, 4)), stop=(b in (3, 5)),
                                 skip_group_check=True)
            aggT = wrk.tile([128, 768], f16, tag="aggT_sb")
            nc.scalar.copy(aggT[:], aggT_p[:])

            # ---- conv = lrelu(agg @ Wk, 0.1) ; out = conv @ W_out + x ----
            co_p = psC.tile([P, 240], f32, tag="co")
            for b in range(6):
                nc.tensor.matmul(co_p[0:W, 0:P], wk_sb[:, W * b:W * (b + 1)],
                                 aggT[:, 128 * b:128 * b + P],
                                 start=(b == 0), stop=(b == 5),
                                 skip_group_check=True)
            convL = wrk.tile([W, P], f16, tag="convL")
            nc.scalar.activation(convL[:], co_p[0:W, 0:P], AF.Prelu, bias=0.0,
                                 scale=1.0, alpha=NEG_IN)
            nc.tensor.matmul(co_p[:, P:P + 128], convL[:], w_out_sb[:],
                             start=True, stop=False, skip_group_check=True)
            # identity add on the PE: accumulate xc into the same psum group
            # via an identity-stationary copy-matmul, then DMA from PSUM.
            nc.tensor.matmul(co_p[:, P:P + 128], id_sb[0:P, 0:P],
                             xc_all[0:P, C * t:C * t + C],
                             start=False, stop=True, skip_group_check=True)
            out_sb = wrk.tile([P, C], f32, tag="out_sb")
            nc.scalar.copy(out_sb[:], co_p[:, P:P + 128])
            cnt = min(TS, NPC - TS * t)
            nc.sync.dma_start(y[TS * t:TS * t + cnt, :], out_sb[0:cnt, :])

    nc.compile()
    return nc


def _expected_src_dst():
    i = np.arange(N)
    offs = np.arange(-WIN, WIN + 1)
    j = i[:, None] + offs[None, :]
    valid = ((j // L) == (i[:, None] // L)) & (j >= 0) & (j < N)
    j = np.where(valid, j, i[:, None])
    dst = np.repeat(i, offs.size).astype(np.int32)
    src = j.reshape(-1).astype(np.int32)
    return src, dst


def _host_inputs(x, pos, ori, W_in, Ws0, bs0, Wk, W_out):
    xf = np.ascontiguousarray(x.reshape(N, C), np.float32)
    pos = np.asarray(pos, np.float32)
    ori = np.asarray(ori, np.float32)
    f16 = np.float16

    # shared weights / constants
    WS = np.zeros((136, K17 * KC), np.float32)
    for k in range(K17):
        s = _sidx(k)
        WS[8 * k:8 * k + 7, KC * k:KC * (k + 1)] = Ws0[s]
        WS[8 * k + 7, KC * k:KC * (k + 1)] = bs0[s]
    # pair-duplicate columns: WS2[:, 48k + 2c + j] = WS[:, 24k + c]
    WS2 = np.repeat(WS, 2, axis=1)
    wk_p = np.zeros((128, 6 * W), np.float32)
    for b in range(6):
        wk_p[:, W * b:W * (b + 1)] = Wk[128 * b:128 * (b + 1), :]
    shifts = np.zeros((128, K17 * TS), np.float32)
    for k in range(K17):
        for p in range(TS):
            shifts[p + k, TS * k + p] = 1.0
    common = dict(
        w_in=W_in.astype(f16),
        ws2a=WS2[0:128].astype(f16),
        ws2b=WS2[128:136].astype(f16),
        wk_p=wk_p.astype(f16),
        w_out=W_out.astype(f16),
        ident=np.eye(128, dtype=f16),
        shifts=shifts.astype(f16),
    )

    # self-edge compensation: kself[n] = lrelu(rn @ W5[3:6] + b5, 0.2) * ncl
    rn = (ori.reshape(N, 3, 3) ** 2).sum(axis=2)          # [N, 3]
    pself = rn @ np.asarray(Ws0[S_HALF][3:6], np.float32) \
        + np.asarray(bs0[S_HALF], np.float32)             # [N, KC]
    kself_full = np.where(pself >= 0, pself, NEG_K * pself)

    in_maps = []
    for ci in range(NCORES):
        s0 = ci * NPC
        g = s0 - WIN + np.arange(HR)
        # chain-aware zero padding: out-of-chain halo rows get h = 0, so
        # their messages vanish with no explicit masking on device.
        ok = (g // L) == (s0 // L)
        gi = np.clip(g, 0, N - 1)
        x_pad = np.where(ok[:, None], xf[gi], 0.0).astype(np.float32)
        p_pad = np.where(ok[:, None], pos[gi], 0.0).astype(np.float32)
        o_pad = np.where(ok[:, None], ori[gi], 0.0).astype(np.float32)

        jj, pp = np.meshgrid(np.arange(NT), np.arange(128), indexing="ij")
        rows = (TS * jj + pp)            # [NT, 128] all < HR
        # xT_slot: [128(c), (t, p)] transposed slots
        x_sl = x_pad[rows]               # [NT, 128, C]
        xT_slot = np.ascontiguousarray(
            x_sl.transpose(2, 0, 1).reshape(C, NT * 128)).astype(f16)
        # pos: center per slot for fp16 precision; interleave with ori
        p_sl = p_pad[rows]               # [NT, 128, 3]
        ctr = p_sl.mean(axis=1, keepdims=True)
        pog = np.concatenate([p_sl - ctr, o_pad[rows]], axis=2)  # [NT,128,12]
        pog_slot = np.ascontiguousarray(
            pog.transpose(1, 0, 2).reshape(128, NT * 12)).astype(f16)
        # identity (center rows)
        rc = WIN + TS * jj + pp
        okc = rc < HR
        xc_slot = np.where(okc[:, :, None], x_pad[np.minimum(rc, HR - 1)], 0.0)
        xc_slot = xc_slot.transpose(1, 0, 2).reshape(128, NT * C).astype(f16)

        # boundary-count + kself2 (output-node indexed)
        ncl = np.zeros((128, NT), np.float32)
        for t in (0, NT - 1):
            nvalid = min(TS, NPC - TS * t)
            for p in range(nvalid):
                off = (s0 + TS * t + p) % L
                v = ((off + np.arange(-WIN, WIN + 1)) >= 0) & \
                    ((off + np.arange(-WIN, WIN + 1)) < L)
                ncl[p, t] = K17 - v.sum()
        ks = np.zeros((128, NT, KC), np.float32)
        for t in (0, NT - 1):
            nvalid = min(TS, NPC - TS * t)
            rowsn = s0 + TS * t + np.arange(nvalid)
            ks[:nvalid, t, :] = kself_full[rowsn] * ncl[:nvalid, t][:, None]
        ks2 = np.repeat(ks, 2, axis=2)  # duplicate pairs within each KC block
        in_maps.append(dict(
            xT_slot=xT_slot, xc_slot=xc_slot, pog_slot=pog_slot,
            kself2=ks2.reshape(128, NT * 2 * KC).astype(f16),
            **common))
    return in_maps


def kernel(x, pos, seq, ori, W_in, Ws0, bs0, Wk, W_out, src, dst):
    exp_src, exp_dst = _expected_src_dst()
    assert np.array_equal(np.asarray(src), exp_src), "unexpected src graph"
    assert np.array_equal(np.asarray(dst), exp_dst), "unexpected dst graph"

    from concourse.bass_utils import run_bass_kernel_spmd

    if "nc" not in _PROG:
        _PROG["nc"] = _build_program()
    nc = _PROG["nc"]

    in_maps = _host_inputs(np.asarray(x), np.asarray(pos), np.asarray(ori),
                           np.asarray(W_in), np.asarray(Ws0), np.asarray(bs0),
                           np.asarray(Wk), np.asarray(W_out))
    res = run_bass_kernel_spmd(nc, in_maps, list(range(NCORES)))
    out = np.concatenate([res.results[i]["y"] for i in range(NCORES)], axis=0)
    return out.reshape(B, L, C).astype(np.float32)


# revision 56
# speedup vs baseline: 1.2155x; 1.0131x over previous
"""Bass/Trainium2 kernel for nn_BasicBlock_73933567033945 (CDConv / gnn_message_passing).

v2 strategy (graph = fixed +-8 sequence window inside 4 chains, verified at
runtime): shard 8192 nodes across 8 cores (1024 each, half a chain), slot
layout of 128-row halo windows at stride 112.  All matmuls and DVE tensor ops
run in fp16 (fp32 PSUM accumulation); pos is slot-centered on host so fp16
holds precision.  The 17 window shifts are materialized once per core by 17
wide shift-matmuls over all 10 slots (h|pos|ori, 440 cols each).  The
per-edge kernel MLP output is written pair-duplicated (kern2) so the
bilinear kern (x) h product runs in the DVE 2x perf mode.  The (offset,
channel) contraction runs on the PE via PSUM-accumulated transposes followed
by Wk-chunk matmuls, all fp16.  Pure data parallel: no collectives.
"""
import numpy as np

B, L, C = 4, 2048, 128
N = B * L
W = 32
KC = 24
SEQ_L = 11
R = 12.0
WIN = 8
NEG_IN = 0.1
NEG_K = 0.2
NCORES = 8
NPC = N // NCORES          # 1024 nodes per core
TS = 112                   # output nodes per tile
NT = 10                    # tiles per core (9*112 + 16)
HR = 9 * TS + 128          # 1136 halo rows per core
K17 = 2 * WIN + 1          # 17 window offsets
S_HALF = SEQ_L // 2
PH = 44                    # phys cols per slot: h(32) | pos(3) | ori(9)
NBW = NT * PH              # 440: NB cols per k

_PROG = {}


def _sidx(k):
    return int(np.clip(k - WIN, -S_HALF, S_HALF)) + S_HALF


def _build_program():
    import concourse.tile as tile
    from concourse import mybir, bacc
    from concourse.bass_utils import run_bass_kernel_spmd  # noqa: F401 (import check)
    from contextlib import ExitStack

    f32 = mybir.dt.float32
    f16 = mybir.dt.float16
    AF = mybir.ActivationFunctionType
    OP = mybir.AluOpType
    AX = mybir.AxisListType

    nc = bacc.Bacc("TRN2", target_bir_lowering=False, debug=False)

    def din(name, shape, dt=f16):
        return nc.dram_tensor(name, shape, dt, kind="ExternalInput").ap()

    xT_slot = din("xT_slot", [128, NT * 128])        # x transposed per slot, f16
    xc_slot = din("xc_slot", [128, NT * C])          # identity (center rows) f16
    pog_slot = din("pog_slot", [128, NT * 12])       # centered pos(3) | ori(9)
    w_in = din("w_in", [C, W])
    ws2a = din("ws2a", [128, 2 * K17 * KC])
    ws2b = din("ws2b", [8, 2 * K17 * KC])
    wk_p = din("wk_p", [128, 6 * W])
    w_out = din("w_out", [W, C])
    ident = din("ident", [128, 128])
    shifts = din("shifts", [128, K17 * TS])
    kself2 = din("kself2", [128, NT * 2 * KC])
    y = nc.dram_tensor("y", [NPC, C], f32, kind="ExternalOutput").ap()

    P = TS  # 112 active partitions

    with tile.TileContext(nc) as tc, ExitStack() as ctx:
        pers = ctx.enter_context(tc.tile_pool(name="pers", bufs=1))

        def load(ap_in, shape, tag, dt=f16):
            t = pers.tile(shape, dt, tag=tag)
            nc.sync.dma_start(t[:], ap_in)
            return t

        # DMA order matters: pos/ori + shift matrices + identity feed the
        # NBg matmuls that start first; xc is only needed at tile ends.
        phys_g = pers.tile([128, NT * 12], f16, tag="phys_g")
        nc.sync.dma_start(phys_g[:], pog_slot)
        sh_sb = pers.tile([128, K17 * TS], f16, tag="shifts")
        for c0 in range(0, K17 * TS, 5 * TS):
            c1 = min(c0 + 5 * TS, K17 * TS)
            nc.sync.dma_start(sh_sb[:, c0:c1], shifts[:, c0:c1])
        id_sb = load(ident, [128, 128], "ident")
        w_in_sb = load(w_in, [C, W], "w_in")
        xT_all = pers.tile([128, NT * 128], f16, tag="xT_all")
        nc.sync.dma_start(xT_all[:, 0:5 * 128], xT_slot[:, 0:5 * 128])
        nc.sync.dma_start(xT_all[:, 5 * 128:], xT_slot[:, 5 * 128:])
        ws2a_sb = load(ws2a, [128, 2 * K17 * KC], "ws2a")
        ws2b_sb = load(ws2b, [8, 2 * K17 * KC], "ws2b")
        wk_sb = load(wk_p, [128, 6 * W], "wk")
        w_out_sb = load(w_out, [W, C], "w_out")
        ks2_sb = load(kself2, [128, NT * 2 * KC], "kself2")
        xc_all = load(xc_slot, [128, NT * C], "xc_all")

        # dist = sqrt(d2 + eps): eps = 1e-4 keeps rec = 1/dist <= 100 (fp16
        # safe; self-edges have D = 0 so local = 0 regardless) while real
        # edge distances (>= ~0.5) are perturbed by < 1e-3 relative.
        eps_sb = pers.tile([128, 1], f32, tag="eps")
        nc.vector.memset(eps_sb[:], 1e-4)
        phys_h = pers.tile([128, NT * W], f16, tag="phys_h")

        # ---- NBg: 17 pos/ori shift matmuls (independent of Phase A) ------
        NBg = pers.tile([P, K17 * NT * 12], f16, tag="NBg")
        NBh = pers.tile([P, K17 * NT * W], f16, tag="NBh")
        GW = NT * 12   # 120
        HW_ = NT * W   # 320
        with tc.tile_pool(name="pNBg", bufs=3, space="PSUM") as pNBg:
            for k in range(K17):
                nb_p = pNBg.tile([P, GW], f32, tag="nbg_p")
                nc.tensor.matmul(nb_p[:], sh_sb[:, TS * k:TS * (k + 1)],
                                 phys_g[:], start=True, stop=True)
                nc.vector.tensor_copy(NBg[:, GW * k:GW * (k + 1)], nb_p[:])

        # ---------------- Phase A: h = lrelu(lrelu(x) @ W_in) per slot -----
        with tc.tile_pool(name="pA", bufs=2) as pA, \
             tc.tile_pool(name="pAp", bufs=2, space="PSUM") as pAp:
            for j in range(NT):
                xlT = pA.tile([128, 128], f16, tag="xlT")
                nc.scalar.activation(xlT[:], xT_all[:, 128 * j:128 * (j + 1)],
                                     AF.Prelu, bias=0.0, scale=1.0, alpha=NEG_IN)
                hp = pAp.tile([128, W], f32, tag="hp")
                nc.tensor.matmul(hp[:], xlT[:], w_in_sb[:], start=True, stop=True)
                nc.scalar.activation(phys_h[:, W * j:W * (j + 1)], hp[:],
                                     AF.Prelu, bias=0.0, scale=1.0, alpha=NEG_IN)

        # ---- NBh: 17 h shift matmuls ------------------------------------
        with tc.tile_pool(name="pNBh", bufs=3, space="PSUM") as pNBh:
            for k in range(K17):
                nb_p = pNBh.tile([P, HW_], f32, tag="nbh_p")
                nc.tensor.matmul(nb_p[:], sh_sb[:, TS * k:TS * (k + 1)],
                                 phys_h[:], start=True, stop=True)
                dst = NBh[:, HW_ * k:HW_ * (k + 1)]
                if k % 2 == 0:
                    nc.scalar.copy(dst, nb_p[:])
                else:
                    nc.vector.tensor_copy(dst, nb_p[:])

        # ---------------- Phase B: per output tile ------------------------
        wrk = ctx.enter_context(tc.tile_pool(name="wrk", bufs=3))
        tpool = ctx.enter_context(tc.tile_pool(name="tmp", bufs=8))
        psG = ctx.enter_context(tc.tile_pool(name="psG", bufs=2, space="PSUM"))
        psD = ctx.enter_context(tc.tile_pool(name="psD", bufs=1, space="PSUM"))
        psP = ctx.enter_context(tc.tile_pool(name="psP", bufs=1, space="PSUM"))
        psC = ctx.enter_context(tc.tile_pool(name="psC", bufs=1, space="PSUM"))

        for t in range(NT):
            # k-strided views into NBg for slot t (pos at off 0, ori at 3)
            def kview(off, width):
                v = NBg[:].rearrange("p (k j) -> p k j", j=GW)
                return v[:, :, 12 * t + off:12 * t + off + width]

            gc = GW * 8 + 12 * t
            pos_c = NBg[:, gc:gc + 3]       # [P, 3] center pos
            ori_c = NBg[:, gc + 3:gc + 12]  # [P, 9] center frame

            # ---- geometry -> dav [P, (k,8)] fp16 -------------------------
            D = wrk.tile([P, K17 * 3], f16, tag="D")
            Dv = D[:].rearrange("p (k a) -> p k a", a=3)
            nc.vector.tensor_sub(Dv, kview(0, 3),
                                 pos_c.unsqueeze(1).broadcast_to([P, K17, 3]))
            sq = wrk.tile([P, K17 * 3], f16, tag="sq")
            nc.vector.tensor_mul(sq[:], D[:], D[:])
            d2 = wrk.tile([P, K17], f32, tag="d2")
            nc.vector.tensor_reduce(d2[:], sq[:].rearrange("p (k a) -> p k a", a=3),
                                    axis=AX.X, op=OP.add)
            dav = wrk.tile([P, K17 * 8], f16, tag="dav")
            davv = dav[:].rearrange("p (k d) -> p k d", d=8)
            # dist/R into delta slot 6 (sqrt(d2)/R)
            nc.scalar.activation(davv[:, :, 6], d2[:], AF.Sqrt, bias=0.0,
                                 scale=1.0 / (R * R))
            dist = wrk.tile([P, K17], f32, tag="dist")
            nc.scalar.activation(dist[:], d2[:], AF.Sqrt, bias=eps_sb[0:P, 0:1],
                                 scale=1.0)
            rec = wrk.tile([P, K17], f16, tag="rec")
            with nc.allow_low_precision(reason="fp16 direction scale is ok"):
                nc.vector.reciprocal(rec[:], dist[:])
            # local_a = (sum_b Ri[a,b] * D[k,b]) * rec[k]
            lm = wrk.tile([P, K17 * 9], f16, tag="lm")
            lmv = lm[:].rearrange("p (k a b) -> p k a b", a=3, b=3)
            nc.vector.tensor_mul(
                lmv,
                ori_c.rearrange("p (a b) -> p a b", b=3).unsqueeze(1)
                     .broadcast_to([P, K17, 3, 3]),
                D[:].rearrange("p (k b) -> p k b", b=3).unsqueeze(2)
                    .broadcast_to([P, K17, 3, 3]))
            locr = wrk.tile([P, K17 * 3], f16, tag="locr")
            with nc.allow_low_precision(reason="3-term sums, fp16 ok"):
                nc.vector.tensor_reduce(
                    locr[:].rearrange("p (k a) -> p k a", a=3), lmv,
                    axis=AX.X, op=OP.add)
            nc.vector.tensor_mul(
                davv[:, :, 0:3], locr[:].rearrange("p (k a) -> p k a", a=3),
                rec[:].unsqueeze(-1).broadcast_to([P, K17, 3]))
            # ofeat_a = sum_b Ri[a,b] * Rj[a,b]
            ofm = wrk.tile([P, K17 * 9], f16, tag="ofm")
            nc.vector.tensor_mul(
                ofm[:].rearrange("p (k e) -> p k e", e=9), kview(3, 9),
                ori_c.unsqueeze(1).broadcast_to([P, K17, 9]))
            with nc.allow_low_precision(reason="3-term sums, fp16 ok"):
                nc.vector.tensor_reduce(
                    davv[:, :, 3:6],
                    ofm[:].rearrange("p (k a b) -> p k a b", a=3, b=3),
                    axis=AX.X, op=OP.add)
            # out-of-chain halo rows are zero-padded on host, so h_src = 0
            # there and fake-edge messages vanish without any masking.  The
            # dav bias slot (7) is only ever written here, so set it just
            # once per pool buffer rotation.
            if t < 3:
                nc.vector.memset(davv[:, :, 7], 1.0)

            # ---- kern2 = lrelu(dav @ WS2, 0.2), pair-duplicated ----------
            dT_p = psD.tile([128, 224], f16, tag="dT")
            nc.tensor.matmul(dT_p[:, 0:P], dav[:, 0:128], id_sb[0:P, 0:P],
                             is_transpose=True, start=True, stop=False,
                             skip_group_check=True)
            nc.tensor.matmul(dT_p[0:8, P:P + P], dav[:, 128:136], id_sb[0:P, 0:P],
                             is_transpose=True, start=False, stop=True,
                             skip_group_check=True)
            dT = wrk.tile([128, 224], f16, tag="dT_sb")
            nc.scalar.copy(dT[:], dT_p[:])
            W2 = 2 * K17 * KC  # 816
            # psum banks are 512 f32 cols: put k-blocks 0..9 at 0:480 (bank 0)
            # and k-blocks 10..16 at 512:848 (bank 1) to avoid bank crossing.
            pre_p = psP.tile([P, 848], f32, tag="pre")
            nc.tensor.matmul(pre_p[:, 0:480], dT[:, 0:P], ws2a_sb[:, 0:480],
                             start=True, stop=False, skip_group_check=True)
            nc.tensor.matmul(pre_p[:, 512:848], dT[:, 0:P], ws2a_sb[:, 480:W2],
                             start=True, stop=False, skip_group_check=True)
            nc.tensor.matmul(pre_p[:, 0:480], dT[0:8, P:P + P], ws2b_sb[:, 0:480],
                             start=False, stop=True, skip_group_check=True)
            nc.tensor.matmul(pre_p[:, 512:848], dT[0:8, P:P + P], ws2b_sb[:, 480:W2],
                             start=False, stop=True, skip_group_check=True)
            kern2 = wrk.tile([P, W2], f16, tag="kern2")
            nc.scalar.activation(kern2[:, 0:480], pre_p[:, 0:480], AF.Prelu,
                                 bias=0.0, scale=1.0, alpha=NEG_K)
            nc.scalar.activation(kern2[:, 480:W2], pre_p[:, 512:848], AF.Prelu,
                                 bias=0.0, scale=1.0, alpha=NEG_K)
            # self-edge compensation (host-precomputed, pair-duplicated);
            # nonzero only at chain ends, which land in tiles 0 and 9
            if t in (0, NT - 1):
                K8 = 2 * KC * 8
                nc.vector.tensor_add(kern2[:, K8:K8 + 2 * KC],
                                     kern2[:, K8:K8 + 2 * KC],
                                     ks2_sb[0:P, 2 * KC * t:2 * KC * (t + 1)])

            # ---- bilinear + PE transpose-accumulate ----------------------
            # gpsimd takes the last 3 offsets (issued first so they finish
            # by the time the PE transpose chain reaches them); DVE does the
            # rest in the 2x packed mode.
            # sum_k kern_k (x) h_k accumulated in normal layout via
            # identity-stationary copy-matmuls (2 per k, split at the psum
            # bank boundary), then ONE transpose set of 6 matmuls.  The
            # same psum tile is reused for the transposed result after the
            # accumulation has been copied out (WAR dep keeps it safe).
            ag_ps = psG.tile([128, 768], f32, tag="agg")
            agg_p = ag_ps[0:P, :]
            for k in range(K17):
                tm = tpool.tile([P, KC * W], f16, tag="tm")
                hv = NBh[:, HW_ * k + W * t:HW_ * k + W * (t + 1)] \
                    .rearrange("p (s two) -> p s two", two=2) \
                    .unsqueeze(1).broadcast_to([P, KC, 16, 2])
                kv = kern2[:, 2 * KC * k:2 * KC * (k + 1)] \
                    .rearrange("p (c two) -> p c two", two=2) \
                    .unsqueeze(2).broadcast_to([P, KC, 16, 2])
                nc.vector.tensor_tensor(
                    tm[:].rearrange("p (c s two) -> p c s two", two=2, s=16),
                    hv, kv, op=OP.mult)
                nc.tensor.matmul(agg_p[:, 0:512], id_sb[0:P, 0:P],
                                 tm[:, 0:512], start=(k == 0), stop=(k == 16),
                                 skip_group_check=True)
                nc.tensor.matmul(agg_p[:, 512:768], id_sb[0:P, 0:P],
                                 tm[:, 512:768], start=(k == 0), stop=(k == 16),
                                 skip_group_check=True)
            agg = wrk.tile([P, 768], f16, tag="agg_sb")
            nc.scalar.copy(agg[:], agg_p[:])
            aggT_p = ag_ps
            for b in range(6):
                nc.tensor.matmul(aggT_p[:, 128 * b:128 * b + P],
                                 agg[:, 128 * b:128 * (b + 1)], id_sb[0:P, 0:P],
                                 start=(b in (0, 4)), stop=(b in (3, 5)),
                                 skip_group_check=True)
            aggT = wrk.tile([128, 768], f16, tag="aggT_sb")
            nc.scalar.copy(aggT[:], aggT_p[:])

            # ---- conv = lrelu(agg @ Wk, 0.1) ; out = conv @ W_out + x ----
            co_p = psC.tile([P, 240], f32, tag="co")
            for b in range(6):
                nc.tensor.matmul(co_p[0:W, 0:P], wk_sb[:, W * b:W * (b + 1)],
                                 aggT[:, 128 * b:128 * b + P],
                                 start=(b == 0), stop=(b == 5),
                                 skip_group_check=True)
            convL = wrk.tile([W, P], f16, tag="convL")
            nc.scalar.activation(convL[:], co_p[0:W, 0:P], AF.Prelu, bias=0.0,
                                 scale=1.0, alpha=NEG_IN)
            nc.tensor.matmul(co_p[:, P:P + 128], convL[:], w_out_sb[:],
                             start=True, stop=False, skip_group_check=True)
            # identity add on the PE: accumulate xc into the same psum group
            # via an identity-stationary copy-matmul, then DMA from PSUM.
            nc.tensor.matmul(co_p[:, P:P + 128], id_sb[0:P, 0:P],
                             xc_all[0:P, C * t:C * t + C],
                             start=False, stop=True, skip_group_check=True)
            out_sb = wrk.tile([P, C], f32, tag="out_sb")
            nc.scalar.copy(out_sb[:], co_p[:, P:P + 128])
            cnt = min(TS, NPC - TS * t)
            nc.sync.dma_start(y[TS * t:TS * t + cnt, :], out_sb[0:cnt, :])

    nc.compile()
    return nc


def _expected_src_dst():
    i = np.arange(N)
    offs = np.arange(-WIN, WIN + 1)
    j = i[:, None] + offs[None, :]
    valid = ((j // L) == (i[:, None] // L)) & (j >= 0) & (j < N)
    j = np.where(valid, j, i[:, None])
    dst = np.repeat(i, offs.size).astype(np.int32)
    src = j.reshape(-1).astype(np.int32)
    return src, dst


def _host_inputs(x, pos, ori, W_in, Ws0, bs0, Wk, W_out):
    xf = np.ascontiguousarray(x.reshape(N, C), np.float32)
    pos = np.asarray(pos, np.float32)
    ori = np.asarray(ori, np.float32)
    f16 = np.float16

    # shared weights / constants
    WS = np.zeros((136, K17 * KC), np.float32)
    for k in range(K17):
        s = _sidx(k)
        WS[8 * k:8 * k + 7, KC * k:KC * (k + 1)] = Ws0[s]
        WS[8 * k + 7, KC * k:KC * (k + 1)] = bs0[s]
    # pair-duplicate columns: WS2[:, 48k + 2c + j] = WS[:, 24k + c]
    WS2 = np.repeat(WS, 2, axis=1)
    wk_p = np.zeros((128, 6 * W), np.float32)
    for b in range(6):
        wk_p[:, W * b:W * (b + 1)] = Wk[128 * b:128 * (b + 1), :]
    shifts = np.zeros((128, K17 * TS), np.float32)
    for k in range(K17):
        for p in range(TS):
            shifts[p + k, TS * k + p] = 1.0
    common = dict(
        w_in=W_in.astype(f16),
        ws2a=WS2[0:128].astype(f16),
        ws2b=WS2[128:136].astype(f16),
        wk_p=wk_p.astype(f16),
        w_out=W_out.astype(f16),
        ident=np.eye(128, dtype=f16),
        shifts=shifts.astype(f16),
    )

    # self-edge compensation: kself[n] = lrelu(rn @ W5[3:6] + b5, 0.2) * ncl
    rn = (ori.reshape(N, 3, 3) ** 2).sum(axis=2)          # [N, 3]
    pself = rn @ np.asarray(Ws0[S_HALF][3:6], np.float32) \
        + np.asarray(bs0[S_HALF], np.float32)             # [N, KC]
    kself_full = np.where(pself >= 0, pself, NEG_K * pself)

    in_maps = []
    for ci in range(NCORES):
        s0 = ci * NPC
        g = s0 - WIN + np.arange(HR)
        # chain-aware zero padding: out-of-chain halo rows get h = 0, so
        # their messages vanish with no explicit masking on device.
        ok = (g // L) == (s0 // L)
        gi = np.clip(g, 0, N - 1)
        x_pad = np.where(ok[:, None], xf[gi], 0.0).astype(np.float32)
        p_pad = np.where(ok[:, None], pos[gi], 0.0).astype(np.float32)
        o_pad = np.where(ok[:, None], ori[gi], 0.0).astype(np.float32)

        jj, pp = np.meshgrid(np.arange(NT), np.arange(128), indexing="ij")
        rows = (TS * jj + pp)            # [NT, 128] all < HR
        # xT_slot: [128(c), (t, p)] transposed slots
        x_sl = x_pad[rows]               # [NT, 128, C]
        xT_slot = np.ascontiguousarray(
            x_sl.transpose(2, 0, 1).reshape(C, NT * 128)).astype(f16)
        # pos: center per slot for fp16 precision; interleave with ori
        p_sl = p_pad[rows]               # [NT, 128, 3]
        ctr = p_sl.mean(axis=1, keepdims=True)
        pog = np.concatenate([p_sl - ctr, o_pad[rows]], axis=2)  # [NT,128,12]
        pog_slot = np.ascontiguousarray(
            pog.transpose(1, 0, 2).reshape(128, NT * 12)).astype(f16)
        # identity (center rows)
        rc = WIN + TS * jj + pp
        okc = rc < HR
        xc_slot = np.where(okc[:, :, None], x_pad[np.minimum(rc, HR - 1)], 0.0)
        xc_slot = xc_slot.transpose(1, 0, 2).reshape(128, NT * C).astype(f16)

        # boundary-count + kself2 (output-node indexed)
        ncl = np.zeros((128, NT), np.float32)
        for t in (0, NT - 1):
            nvalid = min(TS, NPC - TS * t)
            for p in range(nvalid):
                off = (s0 + TS * t + p) % L
                v = ((off + np.arange(-WIN, WIN + 1)) >= 0) & \
                    ((off + np.arange(-WIN, WIN + 1)) < L)
                ncl[p, t] = K17 - v.sum()
        ks = np.zeros((128, NT, KC), np.float32)
        for t in (0, NT - 1):
            nvalid = min(TS, NPC - TS * t)
            rowsn = s0 + TS * t + np.arange(nvalid)
            ks[:nvalid, t, :] = kself_full[rowsn] * ncl[:nvalid, t][:, None]
        ks2 = np.repeat(ks, 2, axis=2)  # duplicate pairs within each KC block
        in_maps.append(dict(
            xT_slot=xT_slot, xc_slot=xc_slot, pog_slot=pog_slot,
            kself2=ks2.reshape(128, NT * 2 * KC).astype(f16),
            **common))
    return in_maps


def kernel(x, pos, seq, ori, W_in, Ws0, bs0, Wk, W_out, src, dst):
    exp_src, exp_dst = _expected_src_dst()
    assert np.array_equal(np.asarray(src), exp_src), "unexpected src graph"
    assert np.array_equal(np.asarray(dst), exp_dst), "unexpected dst graph"

    from concourse.bass_utils import run_bass_kernel_spmd

    if "nc" not in _PROG:
        _PROG["nc"] = _build_program()
    nc = _PROG["nc"]

    in_maps = _host_inputs(np.asarray(x), np.asarray(pos), np.asarray(ori),
                           np.asarray(W_in), np.asarray(Ws0), np.asarray(bs0),
                           np.asarray(Wk), np.asarray(W_out))
    res = run_bass_kernel_spmd(nc, in_maps, list(range(NCORES)))
    out = np.concatenate([res.results[i]["y"] for i in range(NCORES)], axis=0)
    return out.reshape(B, L, C).astype(np.float32)
